# revision 38
# baseline (speedup 1.0000x reference)
"""AttentionBlock (GroupNorm + 8-head self-attention + out-proj + residual) on 8 trn2 cores.

Sharding: core = (batch b, query-half ih).  Each core gets x[b] rolled so that
"its" 1024 query positions are columns 0:1024; K/V are computed over the full
(rolled) L=2048, which is sound because attention and the group-norm statistics
are invariant to a permutation of key/value positions.  Output is the core's
[512, 1024] slice of proj + residual; the host reassembles [4, 512, 2048].

Structure (v2, ACT-saturating):
 - The softmax exp is the hard floor (~133us of ACT time per core: 16.8M
   elements at 1 elem/cycle/lane @1.2GHz + per-inst bubble).  Everything else
   is organized to hide under a continuous stream of 128 exp instructions.
 - exp reads S^T straight from PSUM ([128 keys, 2x512 cols] per 128-key chunk)
   and writes fp8e4m3 to SBUF with the 1/8 scale and a -3 logit bias folded in
   (softmax is shift-invariant; keeps exp < 448 = fp8 max).
 - O = V^T-augmented @ exp runs in fp8 with perf_mode=DoubleRow: 256 keys per
   MM via the [128, 2, *] k-interleave, half the PE time of bf16.  A ones
   column in V^T produces the softmax denominator on PSUM partition 64.
 - All other PE work (remaining qkv tiles, V^T chunks, out-proj) is emitted as
   "spare work" thunks interleaved between attention chunks so the PE never
   bursts long enough to starve ACT.
 - ACT does nothing but exp: group-norm rstd uses Log+Exp (same ACT table set
   as the softmax exp -> exactly one table load), DMAs go on SP/Pool only.
 - PSUM: 4 banks S^T (2 rotations), 2 banks O accum, 2 banks qkv/proj chains.
"""

import sys

sys.path.insert(0, "/opt/trn_rl_repo")

import numpy as np
import ml_dtypes

import concourse.bass as bass
import concourse.mybir as mybir
import concourse.tile as tile
from concourse import bacc
from concourse.vector_clock import ScopedClock, VectorClock
from concourse.bass_utils import run_bass_kernel_spmd

F32 = mybir.dt.float32
BF16 = mybir.dt.bfloat16
FP8 = mybir.dt.float8e4
AX = mybir.AxisListType
OP = mybir.AluOpType
ACTF = mybir.ActivationFunctionType
PMODE = mybir.MatmulPerfMode

B, C, L = 4, 512, 2048
H, D = 8, 64
G, EPS = 32, 1e-5
LQ = L // 2          # queries per core
CT = C // 128        # channel tiles
NJC = L // 128       # key chunks of 128
NJ2 = NJC // 2       # key double-chunks of 256 (DoubleRow granularity)
NIB = LQ // 512      # 512-wide query blocks
DP = D + 2           # V^T head stride: 64 V cols + ones col + zero pad (66, so 8*66 % 16 == 0)
EXP_BIAS = -3.0      # exp(s/8 - 3): softmax-invariant shift, keeps fp8 in range


class _SplitDrainTC(tile.TileContext):
    """Stock exit puts every outstanding proc's wait on one SP Drain; this
    walrus build caps sync-waits per instruction, so spread them over
    single-wait NOPs first."""

    def _drain_and_barrier(self, tick_clock, wait_clock):
        g = tick_clock.global_clock
        for proc in range(len(g)):
            if g[proc] == 0:
                continue
            vc = VectorClock([0] * len(g))
            vc.require_at_least(proc, g[proc])
            nop = self.nc.sync.nop(hint=f"split_drain_{proc}")
            wait_clock.add_sem_waits(nop.ins, ScopedClock({None: vc}))
        self.nc.sync.drain()
        self.nc.all_engine_barrier()
        assert self.sems is not None
        popped = self.nc._tile_sem_poison_stack.pop()
        assert popped is self._sem_poison
        self.nc.clear_and_free_semaphores(list(self.sems.allocated().values()))
        self.nc.all_engine_barrier()


def build_nc(reps: int = 1):
    nc = bacc.Bacc("TRN2", target_bir_lowering=False, num_devices=8)

    xd = nc.declare_dram_parameter("x", [C, L], F32, isOutput=False)
    wqkvT = nc.declare_dram_parameter("wqkvT", [C, 3 * C], BF16, isOutput=False)
    wv8d = nc.declare_dram_parameter("wv8", [128, CT, C], FP8, isOutput=False)
    woutT = nc.declare_dram_parameter("woutT", [C, C], BF16, isOutput=False)
    gnwd = nc.declare_dram_parameter("gnw", [CT, 128], F32, isOutput=False)
    gnbd = nc.declare_dram_parameter("gnb", [CT, 128], F32, isOutput=False)
    boutd = nc.declare_dram_parameter("bout", [128, CT], F32, isOutput=False)
    identd = nc.declare_dram_parameter("ident", [128, 128], F32, isOutput=False)
    yd = nc.declare_dram_parameter("y", [C, LQ], F32, isOutput=True)

    import contextlib

    with _SplitDrainTC(nc) as tc:
        with (
            tc.For_i(0, reps, 1) if reps > 1 else contextlib.nullcontext()
        ), tc.tile_pool(name="persist", bufs=1) as pp:
            x_sb = [pp.tile([128, L], F32, name=f"x{t}", tag=f"x{t}") for t in range(CT)]
            wq_sb = [pp.tile([128, 3 * C], BF16, name=f"wq{t}", tag=f"wq{t}") for t in range(CT)]
            wo_sb = [pp.tile([128, C], BF16, name=f"wo{t}", tag=f"wo{t}") for t in range(CT)]
            nx_sb = [pp.tile([128, L], BF16, name=f"nx{t}", tag=f"nx{t}") for t in range(CT)]
            # fp8 copy of nx (single tensor so DoubleRow APs can span channel
            # tile pairs) + fp8 v-weights: the V^T matmul runs fp8 DoubleRow.
            nx8_sb = pp.tile([128, CT, L], FP8, name="nx8", tag="nx8")
            wv8_sb = pp.tile([128, CT, C], FP8, name="wv8", tag="wv8")
            q_sb = [pp.tile([128, LQ], BF16, name=f"q{t}", tag=f"q{t}") for t in range(CT)]
            k_sb = [pp.tile([128, L], BF16, name=f"k{t}", tag=f"k{t}") for t in range(CT)]
            # V^T double-chunks: [key-in-chunk, ko, head*DP + d]; d=64 is the
            # ones column (denominator), d=65 zero padding.
            vt_sb = [
                pp.tile([128, 2, H, DP], FP8, name=f"vt{j}", tag=f"vt{j}")
                for j in range(NJ2)
            ]
            oh_sb = [pp.tile([128, LQ], F32, name=f"oh{t}", tag=f"oh{t}") for t in range(CT)]
            ohb_sb = [pp.tile([128, LQ], BF16, name=f"ohb{t}", tag=f"ohb{t}") for t in range(CT)]
            gnw_sb = pp.tile([CT, 128], F32, name="gnw", tag="gnw")
            gnb_sb = pp.tile([CT, 128], F32, name="gnb", tag="gnb")
            bout_sb = pp.tile([128, CT], F32, name="bout", tag="bout")
            ones_sb = pp.tile([128, 64], BF16, name="ones", tag="ones")
            nc.vector.memset(ones_sb[:], 1.0)
            ident_sb = pp.tile([128, 128], F32, name="ident", tag="ident")
            sparam_sb = pp.tile([128, 3, CT], F32, name="sparam", tag="sparam")
            ebias_sb = pp.tile([128, 1], F32, name="ebias", tag="ebias")
            nc.vector.memset(ebias_sb[:], EXP_BIAS)

            # ---------------- input DMA (SP + Pool only; ACT stays clean) ----
            nc.gpsimd.dma_start(ident_sb[:], identd[:])
            nc.gpsimd.dma_start(gnw_sb[:], gnwd[:])
            nc.gpsimd.dma_start(gnb_sb[:], gnbd[:])
            nc.gpsimd.dma_start(bout_sb[:], boutd[:])
            xdma_engs = [nc.sync, nc.gpsimd, nc.scalar]
            di = 0
            for t in range(CT):
                for sg in range(4):
                    csl = slice(512 * sg, 512 * sg + 512)
                    xdma_engs[di % 3].dma_start(
                        x_sb[t][:, csl], xd[128 * t : 128 * t + 128, csl]
                    )
                    di += 1
            dma_engs = [nc.sync, nc.gpsimd]
            for t in range(CT):
                for half in range(2):
                    wsl = slice(768 * half, 768 * half + 768)
                    dma_engs[di % 2].dma_start(
                        wq_sb[t][:, wsl], wqkvT[128 * t : 128 * t + 128, wsl]
                    )
                    di += 1
                dma_engs[di % 2].dma_start(
                    wo_sb[t][:], woutT[128 * t : 128 * t + 128, :]
                )
                di += 1
            nc.sync.dma_start(wv8_sb[:], wv8d[:])

            # ---------------- group norm statistics ----------------
            with (
                tc.tile_pool(name="gtmp", bufs=2) as gp,
                tc.tile_pool(name="gps", bufs=2, space="PSUM") as gpp,
            ):
                # stats_all col t = channel-mean(tile t), col 32+t = channel-var:
                # after PE transpose, means land on partitions 0..3 and vars on
                # 32..35 (engine APs may only start at partition 0/32/64/96).
                stats_all = gp.tile([128, 36], F32, name="stats_all", tag="stats_all")
                nc.vector.memset(stats_all[:], 0.0)
                for t in range(CT):
                    st6 = gp.tile([128, 4, 6], F32, name="st6", tag="st6")
                    for sg in range(4):
                        nc.vector.bn_stats(
                            out=st6[:, sg, :],
                            in_=x_sb[t][:, 512 * sg : 512 * sg + 512],
                        )
                    sa = stats_all[:]
                    mv_out = bass.AP(
                        tensor=sa.tensor, offset=sa.offset + t, ap=[sa.ap[0], [32, 2]]
                    )
                    nc.vector.bn_aggr(out=mv_out, in_=st6[:])

                st_ps = gpp.tile([36, 128], F32, name="st_ps", tag="st_ps")
                nc.tensor.transpose(st_ps[:], stats_all[:], ident_sb[:])
                statsT = gp.tile([36, 128], F32, name="statsT", tag="statsT")
                nc.vector.tensor_copy(statsT[:], st_ps[:])

                mred = gp.tile([4, 8], F32, name="mred", tag="mred")
                nc.vector.tensor_reduce(
                    out=mred[:],
                    in_=statsT[0:4, :].rearrange("p (g s) -> p g s", s=16),
                    axis=AX.X,
                    op=OP.add,
                )
                vred = gp.tile([4, 8], F32, name="vred", tag="vred")
                nc.vector.tensor_reduce(
                    out=vred[:],
                    in_=statsT[32:36, :].rearrange("p (g s) -> p g s", s=16),
                    axis=AX.X,
                    op=OP.add,
                )
                sq = gp.tile([4, 128], F32, name="sq", tag="sq")
                nc.vector.tensor_mul(sq[:], statsT[0:4, :], statsT[0:4, :])
                sqred = gp.tile([4, 8], F32, name="sqred", tag="sqred")
                nc.vector.tensor_reduce(
                    out=sqred[:],
                    in_=sq[:].rearrange("p (g s) -> p g s", s=16),
                    axis=AX.X,
                    op=OP.add,
                )
                mg = gp.tile([4, 8], F32, name="mg", tag="mg")
                nc.vector.tensor_scalar_mul(mg[:], mred[:], 1.0 / 16)
                # vg = red_var/16 + sqred/16 - mg^2
                vg = gp.tile([4, 8], F32, name="vg", tag="vg")
                nc.vector.tensor_scalar_mul(vg[:], vred[:], 1.0 / 16)
                nc.vector.scalar_tensor_tensor(
                    out=vg[:],
                    in0=sqred[:],
                    scalar=1.0 / 16,
                    in1=vg[:],
                    op0=OP.mult,
                    op1=OP.add,
                )
                mg2 = gp.tile([4, 8], F32, name="mg2", tag="mg2")
                nc.vector.tensor_mul(mg2[:], mg[:], mg[:])
                nc.vector.tensor_sub(vg[:], vg[:], mg2[:])
                # rstd = (vg + eps)^-0.5 = exp(-0.5*ln(vg + eps)); Log and Exp
                # share one ACT table set with the softmax exp, so the kernel
                # pays exactly one table load.
                epst = gp.tile([4, 1], F32, name="epst", tag="epst")
                nc.vector.memset(epst[:], EPS)
                lvg = gp.tile([4, 8], F32, name="lvg", tag="lvg")
                nc.scalar.activation(out=lvg[:], in_=vg[:], func=ACTF.Ln, bias=epst[:])
                nc.scalar.activation(out=vg[:], in_=lvg[:], func=ACTF.Exp, scale=-0.5)

                # broadcast group -> channels: [4, 8] -> [4, 128]
                def bcast16(src):
                    a = src.ap
                    return bass.AP(
                        tensor=src.tensor, offset=src.offset, ap=[a[0], a[1], [0, 16]]
                    )

                rstd_bc = gp.tile([4, 128], F32, name="rstd_bc", tag="rstd_bc")
                nc.vector.tensor_copy(
                    rstd_bc[:].rearrange("p (g s) -> p g s", s=16), bcast16(vg[:])
                )
                mg_bc = gp.tile([4, 128], F32, name="mg_bc", tag="mg_bc")
                nc.vector.tensor_copy(
                    mg_bc[:].rearrange("p (g s) -> p g s", s=16), bcast16(mg[:])
                )
                s2 = gp.tile([4, 128], F32, name="s2", tag="s2")
                nc.vector.tensor_mul(s2[:], rstd_bc[:], gnw_sb[0:4, :])
                s1 = gp.tile([4, 128], F32, name="s1", tag="s1")
                nc.vector.reciprocal(out=s1[:], in_=s2[:])
                nc.vector.tensor_mul(s1[:], s1[:], gnb_sb[0:4, :])
                nc.vector.tensor_sub(s1[:], mg_bc[:], s1[:])

                # third column: -(s1*s2), the bias form ACT's activation needs
                # for nx = x*s2 + (-s1*s2)
                s12 = gp.tile([4, 128], F32, name="s12", tag="s12")
                nc.vector.scalar_tensor_tensor(
                    out=s12[:], in0=s1[:], scalar=-1.0, in1=s2[:],
                    op0=OP.mult, op1=OP.mult,
                )
                sp_ps = gpp.tile([128, 3, CT], F32, name="sp_ps", tag="sp_ps")
                nc.tensor.transpose(sp_ps[:, 0, :], s1[:], ident_sb[0:4, 0:4])
                nc.tensor.transpose(sp_ps[:, 1, :], s2[:], ident_sb[0:4, 0:4])
                nc.tensor.transpose(sp_ps[:, 2, :], s12[:], ident_sb[0:4, 0:4])
                nc.vector.tensor_copy(sparam_sb[:], sp_ps[:])

            # group-norm apply: nx = (x - s1) * s2, cast to bf16 (+ fp8 for V).
            # Tile 3 goes through ACT (idle here) so the DVE finishes sooner.
            for t in range(3):
                nc.vector.tensor_scalar(
                    out=nx_sb[t][:],
                    in0=x_sb[t][:],
                    scalar1=sparam_sb[:, 0, t : t + 1],
                    scalar2=sparam_sb[:, 1, t : t + 1],
                    op0=OP.subtract,
                    op1=OP.mult,
                )
            nc.scalar.activation(
                out=nx_sb[3][:],
                in_=x_sb[3][:],
                func=ACTF.Identity,
                scale=sparam_sb[:, 1, 3:4],
                bias=sparam_sb[:, 2, 3:4],
            )
            def emit_nx8():
                with nc.allow_low_precision(reason="fp8 V-path intended"):
                    for t in range(CT):
                        nc.vector.tensor_scalar(
                            out=nx8_sb[:, t, :],
                            in0=x_sb[t][:],
                            scalar1=sparam_sb[:, 0, t : t + 1],
                            scalar2=sparam_sb[:, 1, t : t + 1],
                            op0=OP.subtract,
                            op1=OP.mult,
                        )

            # ---------------- attention + interleaved qkv/proj ----------------
            with (
                tc.tile_pool(name="psS", bufs=2, space="PSUM") as pS,
                tc.tile_pool(name="psO", bufs=1, space="PSUM") as pO,
                tc.tile_pool(name="psW", bufs=2, space="PSUM") as pW,
                tc.tile_pool(name="expp", bufs=3) as ep,
                tc.tile_pool(name="rcpp", bufs=2) as rp,
            ):
                # Spare work is sliced into single-matmul units so the PE
                # stream between attention chunks never bursts long enough to
                # delay the next S^T matmul (which gates the exp stream).
                def chain_units(tag, mm_args, finish, n=CT, perf_mode=None):
                    cell = {}
                    units = []

                    def mk(c):
                        def u():
                            if c == 0:
                                cell["ps"] = pW.tile([128, 512], F32, name="w", tag="w")
                            lhsT, rhs = mm_args(c)
                            nc.tensor.matmul(
                                cell["ps"][:], lhsT, rhs,
                                start=(c == 0), stop=(c == n - 1),
                                perf_mode=perf_mode,
                            )
                            if c == n - 1:
                                finish(cell["ps"])
                        return u

                    return [(tag, mk(c)) for c in range(n)]

                def k_units(t, nb):
                    def fin(ps):
                        nc.vector.tensor_copy(
                            k_sb[t][:, 512 * nb : 512 * nb + 512], ps[:]
                        )
                    return chain_units(
                        ("k", t, nb),
                        lambda c: (
                            wq_sb[c][:, C + 128 * t : C + 128 * t + 128],
                            nx_sb[c][:, 512 * nb : 512 * nb + 512],
                        ),
                        fin,
                    )

                def q_units(t, nb):
                    def fin(ps):
                        nc.vector.tensor_copy(
                            q_sb[t][:, 512 * nb : 512 * nb + 512], ps[:]
                        )
                    return chain_units(
                        ("q", t, nb),
                        lambda c: (
                            wq_sb[c][:, 128 * t : 128 * t + 128],
                            nx_sb[c][:, 512 * nb : 512 * nb + 512],
                        ),
                        fin,
                    )

                def vt_units(j2, ko):
                    lt = 2 * j2 + ko

                    def fin(ps):
                        with nc.allow_low_precision(reason="fp8 attention intended"):
                            if ko == 0:
                                nc.vector.memset(vt_sb[j2][:, :, :, D : D + 1], 1.0)
                                nc.vector.memset(vt_sb[j2][:, :, :, D + 1 : D + 2], 0.0)
                            nc.vector.tensor_copy(
                                vt_sb[j2][:, ko, :, 0:D],
                                ps[:].rearrange("p (h d) -> p h d", d=D),
                            )

                    # fp8 DoubleRow over channel-tile pairs: 2 MMs instead of 4
                    return chain_units(
                        ("vt", j2),
                        lambda u: (
                            nx8_sb[:, 2 * u : 2 * u + 2, 128 * lt : 128 * lt + 128],
                            wv8_sb[:, 2 * u : 2 * u + 2, :],
                        ),
                        fin,
                        n=2,
                        perf_mode=PMODE.DoubleRow,
                    )

                def proj_units(t, ib):
                    sl = slice(512 * ib, 512 * ib + 512)

                    def fin(ps):
                        # y = (proj + b_out) + residual, fused; reuse oh as staging
                        nc.vector.scalar_tensor_tensor(
                            out=oh_sb[t][:, sl],
                            in0=ps[:],
                            scalar=bout_sb[:, t : t + 1],
                            in1=x_sb[t][:, sl],
                            op0=OP.add,
                            op1=OP.add,
                        )
                        nc.sync.dma_start(
                            yd[128 * t : 128 * t + 128, sl], oh_sb[t][:, sl]
                        )

                    return chain_units(
                        ("proj", ib),
                        lambda c: (
                            wo_sb[c][:, 128 * t : 128 * t + 128],
                            ohb_sb[c][:, sl],
                        ),
                        fin,
                    )

                # prefix: just enough for (ib0, pair0, jc0) to start; the fp8
                # nx copy comes after so its DVE work doesn't delay the first
                # k/q PSUM->SBUF copies (and with them the first exp).
                for _, u in k_units(0, 0):
                    u()
                for _, u in q_units(0, 0):
                    u()
                emit_nx8()

                # spare-work queue (ordered; consumed between attention chunks;
                # `ensure` pulls a specific chain's remaining units just-in-time)
                work = []
                # interleave the k0 chunks among the vt chunks so each is a
                # little ahead of its just-in-time `ensure` point in pair 0
                work += vt_units(0, 0) + vt_units(0, 1)
                work += vt_units(1, 0) + vt_units(1, 1) + k_units(0, 1)
                work += vt_units(2, 0) + vt_units(2, 1)
                work += vt_units(3, 0) + vt_units(3, 1) + k_units(0, 2)
                work += vt_units(4, 0) + vt_units(4, 1)
                work += vt_units(5, 0) + vt_units(5, 1) + k_units(0, 3)
                work += vt_units(6, 0) + vt_units(6, 1)
                work += vt_units(7, 0) + vt_units(7, 1)
                for t in range(1, CT):
                    for nb in range(4):
                        work += k_units(t, nb)
                    work += q_units(t, 0)
                for t in range(CT):
                    work += q_units(t, 1)

                def emit_spare(n):
                    for _ in range(n):
                        if work:
                            work.pop(0)[1]()

                def ensure(tag):
                    i = 0
                    while i < len(work):
                        if work[i][0] == tag:
                            work.pop(i)[1]()
                        else:
                            i += 1

                for ib in range(NIB):
                    qsl = slice(512 * ib, 512 * ib + 512)
                    if ib == NIB - 1:
                        for t in range(CT):
                            ensure(("q", t, ib))
                    for pair in range(H // 2):
                        ensure(("q", pair, ib))
                        kt, qt = k_sb[pair], q_sb[pair]
                        ops = pO.tile([128, 2, 512], F32, name="O", tag="O")
                        for j2 in range(NJ2):
                            et = ep.tile([128, 2, 1024], FP8, name="exp", tag="exp")
                            for ko in range(2):
                                jc = 2 * j2 + ko
                                ensure(("k", pair, jc // 4))
                                slots = pS.tile([128, 2, 512], F32, name="S", tag="S")
                                nc.tensor.matmul(
                                    slots[:, 0, :],
                                    kt[0:64, 128 * jc : 128 * jc + 128],
                                    qt[0:64, qsl],
                                    start=True,
                                    stop=True,
                                )
                                nc.tensor.matmul(
                                    slots[:, 1, :],
                                    kt[64:128, 128 * jc : 128 * jc + 128],
                                    qt[64:128, qsl],
                                    start=True,
                                    stop=True,
                                )
                                with nc.allow_low_precision(reason="fp8 softmax intended"):
                                    nc.scalar.activation(
                                        out=et[:, ko, :],
                                        in_=slots[:].rearrange("p a b -> p (a b)"),
                                        func=ACTF.Exp,
                                        scale=float(D) ** -0.5,
                                        bias=ebias_sb[:],
                                    )
                                emit_spare(3 if len(work) > 80 else 2)
                            ensure(("vt", j2))
                            for h01 in range(2):
                                nc.tensor.matmul(
                                    ops[0 : D + 2, h01, :],
                                    vt_sb[j2][:, :, 2 * pair + h01, :],
                                    et[:].rearrange("p a (h n) -> p a h n", n=512)[
                                        :, :, h01, :
                                    ],
                                    start=(j2 == 0),
                                    stop=(j2 == NJ2 - 1),
                                    perf_mode=PMODE.DoubleRow,
                                )
                            emit_spare(1)
                        # ---- pair done: copy O out, normalize in place ----
                        # softmax denominators sit on PSUM partition 64 (ones
                        # column of V^T); reciprocal them there, then a K=1
                        # matmul with a ones row at partition 64 broadcasts
                        # 1/denom down to the head's 64 output channels.
                        rcps = rp.tile([128, 2, 512], BF16, name="rcps", tag="rcps")
                        with nc.allow_low_precision(reason="bf16 softmax recip"):
                            nc.vector.reciprocal(
                                out=rcps[64:65, :, :], in_=ops[64:65, :, :]
                            )
                        for h01 in range(2):
                            nc.vector.tensor_copy(
                                oh_sb[pair][64 * h01 : 64 * h01 + 64, qsl],
                                ops[0:64, h01, :],
                            )
                        bps = pW.tile([128, 512], F32, name="w", tag="w")
                        for h01 in range(2):
                            nc.tensor.matmul(
                                bps[64 * h01 : 64 * h01 + 64, :],
                                ones_sb[64:65, 0:64],
                                rcps[64:65, h01, :],
                                start=True,
                                stop=True,
                            )
                        nc.vector.tensor_mul(
                            ohb_sb[pair][:, qsl], oh_sb[pair][:, qsl], bps[:]
                        )
                    # ---- ib done: queue (or emit) its projection ----
                    if ib < NIB - 1:
                        for t in range(CT):
                            work += proj_units(t, ib)
                    else:
                        emit_spare(len(work))  # drain any stragglers
                        for t in range(CT):
                            for _, u in proj_units(t, ib):
                                u()

    nc.compile()
    return nc


_NC_CACHE = None


def _get_nc():
    global _NC_CACHE
    if _NC_CACHE is None:
        _NC_CACHE = build_nc()
    return _NC_CACHE


def _host_inputs(x, gn_w, gn_b, w_qkv, w_out, b_out):
    w_qkvT = np.ascontiguousarray(w_qkv.T).astype(ml_dtypes.bfloat16)
    w_outT = np.ascontiguousarray(w_out.T).astype(ml_dtypes.bfloat16)
    wv8 = np.ascontiguousarray(
        np.asarray(w_qkv[2 * C : 3 * C, :].T, np.float32)
        .reshape(CT, 128, C)
        .transpose(1, 0, 2)
    ).astype(ml_dtypes.float8_e4m3fn)
    ident = np.eye(128, dtype=np.float32)
    shared = {
        "wqkvT": w_qkvT,
        "wv8": wv8,
        "woutT": w_outT,
        "gnw": np.ascontiguousarray(gn_w.reshape(CT, 128), np.float32),
        "gnb": np.ascontiguousarray(gn_b.reshape(CT, 128), np.float32),
        "bout": np.ascontiguousarray(b_out.reshape(CT, 128).T, np.float32),
        "ident": ident,
    }
    in_maps = []
    for core in range(8):
        b, ih = core // 2, core % 2
        xb = np.asarray(x[b], np.float32)
        if ih:
            xb = np.concatenate([xb[:, LQ:], xb[:, :LQ]], axis=1)
        in_maps.append({"x": np.ascontiguousarray(xb), **shared})
    return in_maps


def kernel(x, gn_w, gn_b, w_qkv, w_out, b_out):
    nc = _get_nc()
    in_maps = _host_inputs(
        np.asarray(x), np.asarray(gn_w), np.asarray(gn_b),
        np.asarray(w_qkv), np.asarray(w_out), np.asarray(b_out),
    )
    res = run_bass_kernel_spmd(nc, in_maps, list(range(8)))
    y = np.empty((B, C, L), np.float32)
    for core in range(8):
        b, ih = core // 2, core % 2
        y[b][:, ih * LQ : (ih + 1) * LQ] = res.results[core]["y"]
    return y


# revision 44
# speedup vs baseline: 1.6556x; 1.6556x over previous
"""AttentionBlock (GroupNorm + 8-head self-attention + out-proj + residual) on 8 trn2 cores.

Sharding: core = (batch b, query-half ih).  Each core gets x[b] rolled so that
"its" 1024 query positions are columns 0:1024; K/V are computed over the full
(rolled) L=2048, which is sound because attention and the group-norm statistics
are invariant to a permutation of key/value positions.  Output is the core's
[512, 1024] slice of proj + residual; the host reassembles [4, 512, 2048].

Structure (v2, ACT-saturating):
 - The softmax exp is the hard floor (~133us of ACT time per core: 16.8M
   elements at 1 elem/cycle/lane @1.2GHz + per-inst bubble).  Everything else
   is organized to hide under a continuous stream of 128 exp instructions.
 - exp reads S^T straight from PSUM ([128 keys, 2x512 cols] per 128-key chunk)
   and writes fp8e4m3 to SBUF with the 1/8 scale and a -3 logit bias folded in
   (softmax is shift-invariant; keeps exp < 448 = fp8 max).
 - O = V^T-augmented @ exp runs in fp8 with perf_mode=DoubleRow: 256 keys per
   MM via the [128, 2, *] k-interleave, half the PE time of bf16.  A ones
   column in V^T produces the softmax denominator on PSUM partition 64.
 - All other PE work (remaining qkv tiles, V^T chunks, out-proj) is emitted as
   "spare work" thunks interleaved between attention chunks so the PE never
   bursts long enough to starve ACT.
 - ACT does nothing but exp: group-norm rstd uses Log+Exp (same ACT table set
   as the softmax exp -> exactly one table load), DMAs go on SP/Pool only.
 - PSUM: 4 banks S^T (2 rotations), 2 banks O accum, 2 banks qkv/proj chains.
"""

import sys

sys.path.insert(0, "/opt/trn_rl_repo")

import numpy as np
import ml_dtypes

import concourse.bass as bass
import concourse.mybir as mybir
import concourse.tile as tile
from concourse import bacc
from concourse.vector_clock import ScopedClock, VectorClock
from concourse.bass_utils import run_bass_kernel_spmd

F32 = mybir.dt.float32
BF16 = mybir.dt.bfloat16
FP8 = mybir.dt.float8e4
AX = mybir.AxisListType
OP = mybir.AluOpType
ACTF = mybir.ActivationFunctionType
PMODE = mybir.MatmulPerfMode

B, C, L = 4, 512, 2048
H, D = 8, 64
G, EPS = 32, 1e-5
LQ = L // 2          # queries per core
CT = C // 128        # channel tiles
NJC = L // 128       # key chunks of 128
NJ2 = NJC // 2       # key double-chunks of 256 (DoubleRow granularity)
NIB = LQ // 512      # 512-wide query blocks
DP = D + 2           # V^T head stride: 64 V cols + ones col + zero pad (66, so 8*66 % 16 == 0)
EXP_BIAS = -3.0      # exp(s/8 - 3): softmax-invariant shift, keeps fp8 in range


class _SplitDrainTC(tile.TileContext):
    """Stock exit puts every outstanding proc's wait on one SP Drain; this
    walrus build caps sync-waits per instruction, so spread them over
    single-wait NOPs first."""

    def _drain_and_barrier(self, tick_clock, wait_clock):
        g = tick_clock.global_clock
        for proc in range(len(g)):
            if g[proc] == 0:
                continue
            vc = VectorClock([0] * len(g))
            vc.require_at_least(proc, g[proc])
            nop = self.nc.sync.nop(hint=f"split_drain_{proc}")
            wait_clock.add_sem_waits(nop.ins, ScopedClock({None: vc}))
        self.nc.sync.drain()
        self.nc.all_engine_barrier()
        assert self.sems is not None
        popped = self.nc._tile_sem_poison_stack.pop()
        assert popped is self._sem_poison
        self.nc.clear_and_free_semaphores(list(self.sems.allocated().values()))
        self.nc.all_engine_barrier()


def build_nc(reps: int = 1):
    nc = bacc.Bacc("TRN2", target_bir_lowering=False, num_devices=8)

    xd = nc.declare_dram_parameter("x", [C, L], F32, isOutput=False)
    wqkvT = nc.declare_dram_parameter("wqkvT", [C, 3 * C], BF16, isOutput=False)
    wv8d = nc.declare_dram_parameter("wv8", [128, CT, C], FP8, isOutput=False)
    woutT = nc.declare_dram_parameter("woutT", [C, C], BF16, isOutput=False)
    gnwd = nc.declare_dram_parameter("gnw", [CT, 128], F32, isOutput=False)
    gnbd = nc.declare_dram_parameter("gnb", [CT, 128], F32, isOutput=False)
    boutd = nc.declare_dram_parameter("bout", [128, CT], F32, isOutput=False)
    identd = nc.declare_dram_parameter("ident", [128, 128], F32, isOutput=False)
    yd = nc.declare_dram_parameter("y", [C, LQ], F32, isOutput=True)

    import contextlib

    with _SplitDrainTC(nc) as tc:
        with (
            tc.For_i(0, reps, 1) if reps > 1 else contextlib.nullcontext()
        ), tc.tile_pool(name="persist", bufs=1) as pp:
            x_sb = [pp.tile([128, L], F32, name=f"x{t}", tag=f"x{t}") for t in range(CT)]
            wq_sb = [pp.tile([128, 3 * C], BF16, name=f"wq{t}", tag=f"wq{t}") for t in range(CT)]
            wo_sb = [pp.tile([128, C], BF16, name=f"wo{t}", tag=f"wo{t}") for t in range(CT)]
            nx_sb = [pp.tile([128, L], BF16, name=f"nx{t}", tag=f"nx{t}") for t in range(CT)]
            # fp8 copy of nx (single tensor so DoubleRow APs can span channel
            # tile pairs) + fp8 v-weights: the V^T matmul runs fp8 DoubleRow.
            nx8_sb = pp.tile([128, CT, L], FP8, name="nx8", tag="nx8")
            wv8_sb = pp.tile([128, CT, C], FP8, name="wv8", tag="wv8")
            q_sb = [pp.tile([128, LQ], BF16, name=f"q{t}", tag=f"q{t}") for t in range(CT)]
            k_sb = [pp.tile([128, L], BF16, name=f"k{t}", tag=f"k{t}") for t in range(CT)]
            # V^T double-chunks: [key-in-chunk, ko, head*DP + d]; d=64 is the
            # ones column (denominator), d=65 zero padding.
            vt_sb = [
                pp.tile([128, 2, H, DP], FP8, name=f"vt{j}", tag=f"vt{j}")
                for j in range(NJ2)
            ]
            oh_sb = [pp.tile([128, LQ], F32, name=f"oh{t}", tag=f"oh{t}") for t in range(CT)]
            ohb_sb = [pp.tile([128, LQ], BF16, name=f"ohb{t}", tag=f"ohb{t}") for t in range(CT)]
            gnw_sb = pp.tile([CT, 128], F32, name="gnw", tag="gnw")
            gnb_sb = pp.tile([CT, 128], F32, name="gnb", tag="gnb")
            bout_sb = pp.tile([128, CT], F32, name="bout", tag="bout")
            ones_sb = pp.tile([128, 64], BF16, name="ones", tag="ones")
            nc.vector.memset(ones_sb[:], 1.0)
            ident_sb = pp.tile([128, 128], F32, name="ident", tag="ident")
            sparam_sb = pp.tile([128, 3, CT], F32, name="sparam", tag="sparam")
            ebias_sb = pp.tile([128, 1], F32, name="ebias", tag="ebias")
            nc.vector.memset(ebias_sb[:], EXP_BIAS)
            dummy_sb = pp.tile([1, 1], F32, name="dummy", tag="dummy")
            nc.vector.memset(dummy_sb[:], 1.0)

            # ---------------- input DMA (SP + Pool only; ACT stays clean) ----
            nc.gpsimd.dma_start(ident_sb[:], identd[:])
            nc.gpsimd.dma_start(gnw_sb[:], gnwd[:])
            nc.gpsimd.dma_start(gnb_sb[:], gnbd[:])
            nc.gpsimd.dma_start(bout_sb[:], boutd[:])
            xdma_engs = [nc.sync, nc.gpsimd, nc.scalar]
            di = 0
            for t in range(CT):
                for sg in range(4):
                    csl = slice(512 * sg, 512 * sg + 512)
                    xdma_engs[di % 3].dma_start(
                        x_sb[t][:, csl], xd[128 * t : 128 * t + 128, csl]
                    )
                    di += 1
            dma_engs = [nc.sync, nc.gpsimd]
            for t in range(CT):
                for half in range(2):
                    wsl = slice(768 * half, 768 * half + 768)
                    dma_engs[di % 2].dma_start(
                        wq_sb[t][:, wsl], wqkvT[128 * t : 128 * t + 128, wsl]
                    )
                    di += 1
                dma_engs[di % 2].dma_start(
                    wo_sb[t][:], woutT[128 * t : 128 * t + 128, :]
                )
                di += 1
            nc.sync.dma_start(wv8_sb[:], wv8d[:])
            # dummy Ln pulls the natural_log_exp table load (1.3us) off the
            # stats critical path: it happens here while ACT is otherwise idle
            nc.scalar.activation(out=dummy_sb[:], in_=dummy_sb[:], func=ACTF.Ln)

            # ---------------- group norm statistics ----------------
            with (
                tc.tile_pool(name="gtmp", bufs=2) as gp,
                tc.tile_pool(name="gps", bufs=2, space="PSUM") as gpp,
            ):
                # stats_all col t = channel-mean(tile t), col 32+t = channel-var:
                # after PE transpose, means land on partitions 0..3 and vars on
                # 32..35 (engine APs may only start at partition 0/32/64/96).
                stats_all = gp.tile([128, 36], F32, name="stats_all", tag="stats_all")
                nc.vector.memset(stats_all[:], 0.0)
                for t in range(CT):
                    st6 = gp.tile([128, 4, 6], F32, name="st6", tag="st6")
                    for sg in range(4):
                        nc.vector.bn_stats(
                            out=st6[:, sg, :],
                            in_=x_sb[t][:, 512 * sg : 512 * sg + 512],
                        )
                    sa = stats_all[:]
                    mv_out = bass.AP(
                        tensor=sa.tensor, offset=sa.offset + t, ap=[sa.ap[0], [32, 2]]
                    )
                    nc.vector.bn_aggr(out=mv_out, in_=st6[:])

                st_ps = gpp.tile([36, 128], F32, name="st_ps", tag="st_ps")
                nc.tensor.transpose(st_ps[:], stats_all[:], ident_sb[:])
                statsT = gp.tile([36, 128], F32, name="statsT", tag="statsT")
                nc.vector.tensor_copy(statsT[:], st_ps[:])

                mred = gp.tile([4, 8], F32, name="mred", tag="mred")
                nc.vector.tensor_reduce(
                    out=mred[:],
                    in_=statsT[0:4, :].rearrange("p (g s) -> p g s", s=16),
                    axis=AX.X,
                    op=OP.add,
                )
                vred = gp.tile([4, 8], F32, name="vred", tag="vred")
                nc.vector.tensor_reduce(
                    out=vred[:],
                    in_=statsT[32:36, :].rearrange("p (g s) -> p g s", s=16),
                    axis=AX.X,
                    op=OP.add,
                )
                sq = gp.tile([4, 128], F32, name="sq", tag="sq")
                nc.vector.tensor_mul(sq[:], statsT[0:4, :], statsT[0:4, :])
                sqred = gp.tile([4, 8], F32, name="sqred", tag="sqred")
                nc.vector.tensor_reduce(
                    out=sqred[:],
                    in_=sq[:].rearrange("p (g s) -> p g s", s=16),
                    axis=AX.X,
                    op=OP.add,
                )
                mg = gp.tile([4, 8], F32, name="mg", tag="mg")
                nc.vector.tensor_scalar_mul(mg[:], mred[:], 1.0 / 16)
                # vg = red_var/16 + sqred/16 - mg^2
                vg = gp.tile([4, 8], F32, name="vg", tag="vg")
                nc.vector.tensor_scalar_mul(vg[:], vred[:], 1.0 / 16)
                nc.vector.scalar_tensor_tensor(
                    out=vg[:],
                    in0=sqred[:],
                    scalar=1.0 / 16,
                    in1=vg[:],
                    op0=OP.mult,
                    op1=OP.add,
                )
                mg2 = gp.tile([4, 8], F32, name="mg2", tag="mg2")
                nc.vector.tensor_mul(mg2[:], mg[:], mg[:])
                nc.vector.tensor_sub(vg[:], vg[:], mg2[:])
                # rstd = (vg + eps)^-0.5 = exp(-0.5*ln(vg + eps)); Log and Exp
                # share one ACT table set with the softmax exp, so the kernel
                # pays exactly one table load.
                epst = gp.tile([4, 1], F32, name="epst", tag="epst")
                nc.vector.memset(epst[:], EPS)
                lvg = gp.tile([4, 8], F32, name="lvg", tag="lvg")
                nc.scalar.activation(out=lvg[:], in_=vg[:], func=ACTF.Ln, bias=epst[:])
                nc.scalar.activation(out=vg[:], in_=lvg[:], func=ACTF.Exp, scale=-0.5)

                # broadcast group -> channels: [4, 8] -> [4, 128]
                def bcast16(src):
                    a = src.ap
                    return bass.AP(
                        tensor=src.tensor, offset=src.offset, ap=[a[0], a[1], [0, 16]]
                    )

                rstd_bc = gp.tile([4, 128], F32, name="rstd_bc", tag="rstd_bc")
                nc.vector.tensor_copy(
                    rstd_bc[:].rearrange("p (g s) -> p g s", s=16), bcast16(vg[:])
                )
                mg_bc = gp.tile([4, 128], F32, name="mg_bc", tag="mg_bc")
                nc.vector.tensor_copy(
                    mg_bc[:].rearrange("p (g s) -> p g s", s=16), bcast16(mg[:])
                )
                s2 = gp.tile([4, 128], F32, name="s2", tag="s2")
                nc.vector.tensor_mul(s2[:], rstd_bc[:], gnw_sb[0:4, :])
                s1 = gp.tile([4, 128], F32, name="s1", tag="s1")
                nc.vector.reciprocal(out=s1[:], in_=s2[:])
                nc.vector.tensor_mul(s1[:], s1[:], gnb_sb[0:4, :])
                nc.vector.tensor_sub(s1[:], mg_bc[:], s1[:])

                # third column: -(s1*s2), the bias form ACT's activation needs
                # for nx = x*s2 + (-s1*s2)
                s12 = gp.tile([4, 128], F32, name="s12", tag="s12")
                nc.vector.scalar_tensor_tensor(
                    out=s12[:], in0=s1[:], scalar=-1.0, in1=s2[:],
                    op0=OP.mult, op1=OP.mult,
                )
                sp_ps = gpp.tile([128, 3, CT], F32, name="sp_ps", tag="sp_ps")
                nc.tensor.transpose(sp_ps[:, 0, :], s1[:], ident_sb[0:4, 0:4])
                nc.tensor.transpose(sp_ps[:, 1, :], s2[:], ident_sb[0:4, 0:4])
                nc.tensor.transpose(sp_ps[:, 2, :], s12[:], ident_sb[0:4, 0:4])
                nc.vector.tensor_copy(sparam_sb[:], sp_ps[:])

            # group-norm apply: nx = (x - s1) * s2, cast to bf16 (+ fp8 for V).
            # Tile 3 goes through ACT (idle here) so the DVE finishes sooner.
            for t in range(3):
                nc.vector.tensor_scalar(
                    out=nx_sb[t][:],
                    in0=x_sb[t][:],
                    scalar1=sparam_sb[:, 0, t : t + 1],
                    scalar2=sparam_sb[:, 1, t : t + 1],
                    op0=OP.subtract,
                    op1=OP.mult,
                )
            nc.scalar.activation(
                out=nx_sb[3][:],
                in_=x_sb[3][:],
                func=ACTF.Identity,
                scale=sparam_sb[:, 1, 3:4],
                bias=sparam_sb[:, 2, 3:4],
            )
            def emit_nx8():
                with nc.allow_low_precision(reason="fp8 V-path intended"):
                    for t in range(CT):
                        nc.vector.tensor_scalar(
                            out=nx8_sb[:, t, :],
                            in0=x_sb[t][:],
                            scalar1=sparam_sb[:, 0, t : t + 1],
                            scalar2=sparam_sb[:, 1, t : t + 1],
                            op0=OP.subtract,
                            op1=OP.mult,
                        )

            # ---------------- attention + interleaved qkv/proj ----------------
            with (
                tc.tile_pool(name="psS", bufs=2, space="PSUM") as pS,
                tc.tile_pool(name="psO", bufs=1, space="PSUM") as pO,
                tc.tile_pool(name="psW", bufs=2, space="PSUM") as pW,
                tc.tile_pool(name="expp", bufs=3) as ep,
                tc.tile_pool(name="rcpp", bufs=2) as rp,
            ):
                # Spare work is sliced into single-matmul units so the PE
                # stream between attention chunks never bursts long enough to
                # delay the next S^T matmul (which gates the exp stream).
                def chain_units(tag, mm_args, finish, n=CT, perf_mode=None):
                    cell = {}
                    units = []

                    def mk(c):
                        def u():
                            if c == 0:
                                cell["ps"] = pW.tile([128, 512], F32, name="w", tag="w")
                            lhsT, rhs = mm_args(c)
                            nc.tensor.matmul(
                                cell["ps"][:], lhsT, rhs,
                                start=(c == 0), stop=(c == n - 1),
                                perf_mode=perf_mode,
                            )
                            if c == n - 1:
                                finish(cell["ps"])
                        return u

                    return [(tag, mk(c)) for c in range(n)]

                def k_units(t, nb):
                    def fin(ps):
                        nc.vector.tensor_copy(
                            k_sb[t][:, 512 * nb : 512 * nb + 512], ps[:]
                        )
                    return chain_units(
                        ("k", t, nb),
                        lambda c: (
                            wq_sb[c][:, C + 128 * t : C + 128 * t + 128],
                            nx_sb[c][:, 512 * nb : 512 * nb + 512],
                        ),
                        fin,
                    )

                def q_units(t, nb):
                    def fin(ps):
                        nc.vector.tensor_copy(
                            q_sb[t][:, 512 * nb : 512 * nb + 512], ps[:]
                        )
                    return chain_units(
                        ("q", t, nb),
                        lambda c: (
                            wq_sb[c][:, 128 * t : 128 * t + 128],
                            nx_sb[c][:, 512 * nb : 512 * nb + 512],
                        ),
                        fin,
                    )

                def vt_units(j2, ko):
                    lt = 2 * j2 + ko

                    def fin(ps):
                        with nc.allow_low_precision(reason="fp8 attention intended"):
                            if ko == 0:
                                nc.vector.memset(vt_sb[j2][:, :, :, D : D + 1], 1.0)
                                nc.vector.memset(vt_sb[j2][:, :, :, D + 1 : D + 2], 0.0)
                            nc.vector.tensor_copy(
                                vt_sb[j2][:, ko, :, 0:D],
                                ps[:].rearrange("p (h d) -> p h d", d=D),
                            )

                    # fp8 DoubleRow over channel-tile pairs: 2 MMs instead of 4
                    return chain_units(
                        ("vt", j2),
                        lambda u: (
                            nx8_sb[:, 2 * u : 2 * u + 2, 128 * lt : 128 * lt + 128],
                            wv8_sb[:, 2 * u : 2 * u + 2, :],
                        ),
                        fin,
                        n=2,
                        perf_mode=PMODE.DoubleRow,
                    )

                def proj_units(t, ib):
                    sl = slice(512 * ib, 512 * ib + 512)

                    def fin(ps):
                        # y = (proj + b_out) + residual, fused; reuse oh as staging
                        nc.vector.scalar_tensor_tensor(
                            out=oh_sb[t][:, sl],
                            in0=ps[:],
                            scalar=bout_sb[:, t : t + 1],
                            in1=x_sb[t][:, sl],
                            op0=OP.add,
                            op1=OP.add,
                        )
                        nc.sync.dma_start(
                            yd[128 * t : 128 * t + 128, sl], oh_sb[t][:, sl]
                        )

                    return chain_units(
                        ("proj", ib),
                        lambda c: (
                            wo_sb[c][:, 128 * t : 128 * t + 128],
                            ohb_sb[c][:, sl],
                        ),
                        fin,
                    )

                # prefix: just enough for (ib0, pair0, jc0) to start; the fp8
                # nx copy comes after so its DVE work doesn't delay the first
                # k/q PSUM->SBUF copies (and with them the first exp).
                for _, u in k_units(0, 0):
                    u()
                for _, u in q_units(0, 0):
                    u()
                emit_nx8()

                # spare-work queue (ordered; consumed between attention chunks;
                # `ensure` pulls a specific chain's remaining units just-in-time)
                work = []
                # interleave the k0 chunks among the vt chunks so each is a
                # little ahead of its just-in-time `ensure` point in pair 0
                work += vt_units(0, 0) + vt_units(0, 1)
                work += vt_units(1, 0) + vt_units(1, 1) + k_units(0, 1)
                work += vt_units(2, 0) + vt_units(2, 1)
                work += vt_units(3, 0) + vt_units(3, 1) + k_units(0, 2)
                work += vt_units(4, 0) + vt_units(4, 1)
                work += vt_units(5, 0) + vt_units(5, 1) + k_units(0, 3)
                work += vt_units(6, 0) + vt_units(6, 1)
                work += vt_units(7, 0) + vt_units(7, 1)
                # remaining chunks in the order the attention loop needs them
                for t in range(1, CT):
                    work += q_units(t, 0) + k_units(t, 0)
                    work += k_units(t, 1) + k_units(t, 2) + k_units(t, 3)
                for t in range(CT):
                    work += q_units(t, 1)

                def emit_spare(n):
                    for _ in range(n):
                        if work:
                            work.pop(0)[1]()

                def ensure(tag):
                    i = 0
                    while i < len(work):
                        if work[i][0] == tag:
                            work.pop(i)[1]()
                        else:
                            i += 1

                for ib in range(NIB):
                    qsl = slice(512 * ib, 512 * ib + 512)
                    if ib == NIB - 1:
                        for t in range(CT):
                            ensure(("q", t, ib))
                    for pair in range(H // 2):
                        ensure(("q", pair, ib))
                        kt, qt = k_sb[pair], q_sb[pair]
                        ops = pO.tile([128, 2, 512], F32, name="O", tag="O")
                        for j2 in range(NJ2):
                            et = ep.tile([128, 2, 1024], FP8, name="exp", tag="exp")
                            for ko in range(2):
                                jc = 2 * j2 + ko
                                ensure(("k", pair, jc // 4))
                                slots = pS.tile([128, 2, 512], F32, name="S", tag="S")
                                nc.tensor.matmul(
                                    slots[:, 0, :],
                                    kt[0:64, 128 * jc : 128 * jc + 128],
                                    qt[0:64, qsl],
                                    start=True,
                                    stop=True,
                                )
                                nc.tensor.matmul(
                                    slots[:, 1, :],
                                    kt[64:128, 128 * jc : 128 * jc + 128],
                                    qt[64:128, qsl],
                                    start=True,
                                    stop=True,
                                )
                                with nc.allow_low_precision(reason="fp8 softmax intended"):
                                    nc.scalar.activation(
                                        out=et[:, ko, :],
                                        in_=slots[:].rearrange("p a b -> p (a b)"),
                                        func=ACTF.Exp,
                                        scale=float(D) ** -0.5,
                                        bias=ebias_sb[:],
                                    )
                                emit_spare(3 if len(work) > 80 else 2)
                            ensure(("vt", j2))
                            for h01 in range(2):
                                nc.tensor.matmul(
                                    ops[0 : D + 2, h01, :],
                                    vt_sb[j2][:, :, 2 * pair + h01, :],
                                    et[:].rearrange("p a (h n) -> p a h n", n=512)[
                                        :, :, h01, :
                                    ],
                                    start=(j2 == 0),
                                    stop=(j2 == NJ2 - 1),
                                    perf_mode=PMODE.DoubleRow,
                                )
                            emit_spare(1)
                        # ---- pair done: copy O out, normalize in place ----
                        # softmax denominators sit on PSUM partition 64 (ones
                        # column of V^T); reciprocal them there, then a K=1
                        # matmul with a ones row at partition 64 broadcasts
                        # 1/denom down to the head's 64 output channels.
                        last = ib == NIB - 1 and pair == H // 2 - 1
                        tail_ps = []
                        if last:
                            # pre-run proj chains c0..c2 under this pair's
                            # normalize; t2/t3 borrow the now-idle S banks and
                            # this pair's bps borrows the O banks, so only the
                            # c3 matmuls + fused adds remain after the last mul
                            emit_spare(len(work))
                            for t in range(CT):
                                if t < 2:
                                    pst = pW.tile([128, 512], F32, name="w", tag="w")
                                    psv = pst[:]
                                else:
                                    pst = pS.tile([128, 2, 512], F32, name="S", tag="S")
                                    psv = pst[:, 0, :]
                                for c in range(CT - 1):
                                    nc.tensor.matmul(
                                        psv,
                                        wo_sb[c][:, 128 * t : 128 * t + 128],
                                        ohb_sb[c][:, qsl],
                                        start=(c == 0),
                                        stop=False,
                                    )
                                tail_ps.append(psv)
                        rcps = rp.tile([128, 2, 512], BF16, name="rcps", tag="rcps")
                        with nc.allow_low_precision(reason="bf16 softmax recip"):
                            nc.vector.reciprocal(
                                out=rcps[64:65, :, :], in_=ops[64:65, :, :]
                            )
                        for h01 in range(2):
                            nc.vector.tensor_copy(
                                oh_sb[pair][64 * h01 : 64 * h01 + 64, qsl],
                                ops[0:64, h01, :],
                            )
                        if last:
                            bpst = pO.tile([128, 2, 512], F32, name="O", tag="O")
                            bps = bpst[0:128, 0, :]
                        else:
                            bpst = pW.tile([128, 512], F32, name="w", tag="w")
                            bps = bpst[:]
                        for h01 in range(2):
                            nc.tensor.matmul(
                                bpst[64 * h01 : 64 * h01 + 64, 0, :]
                                if last
                                else bpst[64 * h01 : 64 * h01 + 64, :],
                                ones_sb[64:65, 0:64],
                                rcps[64:65, h01, :],
                                start=True,
                                stop=True,
                            )
                        nc.vector.tensor_mul(
                            ohb_sb[pair][:, qsl], oh_sb[pair][:, qsl], bps
                        )
                    # ---- ib done: queue (or emit) its projection ----
                    if ib < NIB - 1:
                        for t in range(CT):
                            work += proj_units(t, ib)
                    else:
                        for t in range(CT):
                            nc.tensor.matmul(
                                tail_ps[t],
                                wo_sb[CT - 1][:, 128 * t : 128 * t + 128],
                                ohb_sb[CT - 1][:, qsl],
                                start=False,
                                stop=True,
                            )
                            nc.vector.scalar_tensor_tensor(
                                out=oh_sb[t][:, qsl],
                                in0=tail_ps[t],
                                scalar=bout_sb[:, t : t + 1],
                                in1=x_sb[t][:, qsl],
                                op0=OP.add,
                                op1=OP.add,
                            )
                            nc.sync.dma_start(
                                yd[128 * t : 128 * t + 128, qsl], oh_sb[t][:, qsl]
                            )

    nc.compile()
    return nc


_NC_CACHE = None


def _get_nc():
    global _NC_CACHE
    if _NC_CACHE is None:
        _NC_CACHE = build_nc()
    return _NC_CACHE


def _host_inputs(x, gn_w, gn_b, w_qkv, w_out, b_out):
    w_qkvT = np.ascontiguousarray(w_qkv.T).astype(ml_dtypes.bfloat16)
    w_outT = np.ascontiguousarray(w_out.T).astype(ml_dtypes.bfloat16)
    wv8 = np.ascontiguousarray(
        np.asarray(w_qkv[2 * C : 3 * C, :].T, np.float32)
        .reshape(CT, 128, C)
        .transpose(1, 0, 2)
    ).astype(ml_dtypes.float8_e4m3fn)
    ident = np.eye(128, dtype=np.float32)
    shared = {
        "wqkvT": w_qkvT,
        "wv8": wv8,
        "woutT": w_outT,
        "gnw": np.ascontiguousarray(gn_w.reshape(CT, 128), np.float32),
        "gnb": np.ascontiguousarray(gn_b.reshape(CT, 128), np.float32),
        "bout": np.ascontiguousarray(b_out.reshape(CT, 128).T, np.float32),
        "ident": ident,
    }
    in_maps = []
    for core in range(8):
        b, ih = core // 2, core % 2
        xb = np.asarray(x[b], np.float32)
        if ih:
            xb = np.concatenate([xb[:, LQ:], xb[:, :LQ]], axis=1)
        in_maps.append({"x": np.ascontiguousarray(xb), **shared})
    return in_maps


def kernel(x, gn_w, gn_b, w_qkv, w_out, b_out):
    nc = _get_nc()
    in_maps = _host_inputs(
        np.asarray(x), np.asarray(gn_w), np.asarray(gn_b),
        np.asarray(w_qkv), np.asarray(w_out), np.asarray(b_out),
    )
    res = run_bass_kernel_spmd(nc, in_maps, list(range(8)))
    y = np.empty((B, C, L), np.float32)
    for core in range(8):
        b, ih = core // 2, core % 2
        y[b][:, ih * LQ : (ih + 1) * LQ] = res.results[core]["y"]
    return y


# revision 46
# speedup vs baseline: 1.7513x; 1.0578x over previous
"""AttentionBlock (GroupNorm + 8-head self-attention + out-proj + residual) on 8 trn2 cores.

Sharding: core = (batch b, query-half ih).  Each core gets x[b] rolled so that
"its" 1024 query positions are columns 0:1024; K/V are computed over the full
(rolled) L=2048, which is sound because attention and the group-norm statistics
are invariant to a permutation of key/value positions.  Output is the core's
[512, 1024] slice of proj + residual; the host reassembles [4, 512, 2048].

Structure (v2, ACT-saturating):
 - The softmax exp is the hard floor (~133us of ACT time per core: 16.8M
   elements at 1 elem/cycle/lane @1.2GHz + per-inst bubble).  Everything else
   is organized to hide under a continuous stream of 128 exp instructions.
 - exp reads S^T straight from PSUM ([128 keys, 2x512 cols] per 128-key chunk)
   and writes fp8e4m3 to SBUF with the 1/8 scale and a -3 logit bias folded in
   (softmax is shift-invariant; keeps exp < 448 = fp8 max).
 - O = V^T-augmented @ exp runs in fp8 with perf_mode=DoubleRow: 256 keys per
   MM via the [128, 2, *] k-interleave, half the PE time of bf16.  A ones
   column in V^T produces the softmax denominator on PSUM partition 64.
 - The V^T = nx^T Wv matmuls also run fp8 DoubleRow (256 channels per MM) off
   a dedicated fp8 copy of nx, so pair 0 can feed V^T just-in-time without
   starving the exp stream.
 - All other PE work (remaining qkv tiles, V^T chunks, out-proj) is emitted as
   single-matmul "spare work" units interleaved between attention chunks (a
   tagged queue with just-in-time `ensure` pulls), so the PE never bursts long
   enough to starve ACT.
 - Softmax division: 1/denominator is computed in place on PSUM partition 64,
   then a K=1 matmul against a ones row at partition 64 broadcasts it to the
   head's 64 channels - no cross-partition DMA, normalize is per-pair local.
 - ACT does nothing else: group-norm rstd uses Ln+Exp (same ACT table set as
   the softmax exp; a dummy Ln at kernel start hoists the table load off the
   critical path), one nx tile is applied via ACT activation(Identity) to
   shorten the DVE prefix, and bulk DMAs go on SP/Pool queues.
 - PSUM: 4 banks S^T (2 rotations), 2 banks O accum, 2 banks qkv/proj chains.
"""

import sys

sys.path.insert(0, "/opt/trn_rl_repo")

import numpy as np
import ml_dtypes

import concourse.bass as bass
import concourse.mybir as mybir
import concourse.tile as tile
from concourse import bacc
from concourse.vector_clock import ScopedClock, VectorClock
from concourse.bass_utils import run_bass_kernel_spmd

F32 = mybir.dt.float32
BF16 = mybir.dt.bfloat16
FP8 = mybir.dt.float8e4
AX = mybir.AxisListType
OP = mybir.AluOpType
ACTF = mybir.ActivationFunctionType
PMODE = mybir.MatmulPerfMode

B, C, L = 4, 512, 2048
H, D = 8, 64
G, EPS = 32, 1e-5
LQ = L // 2          # queries per core
CT = C // 128        # channel tiles
NJC = L // 128       # key chunks of 128
NJ2 = NJC // 2       # key double-chunks of 256 (DoubleRow granularity)
NIB = LQ // 512      # 512-wide query blocks
DP = D + 2           # V^T head stride: 64 V cols + ones col + zero pad (66, so 8*66 % 16 == 0)
EXP_BIAS = -3.0      # exp(s/8 - 3): softmax-invariant shift, keeps fp8 in range


class _SplitDrainTC(tile.TileContext):
    """Stock exit puts every outstanding proc's wait on one SP Drain; this
    walrus build caps sync-waits per instruction, so spread them over
    single-wait NOPs first."""

    def _drain_and_barrier(self, tick_clock, wait_clock):
        g = tick_clock.global_clock
        for proc in range(len(g)):
            if g[proc] == 0:
                continue
            vc = VectorClock([0] * len(g))
            vc.require_at_least(proc, g[proc])
            nop = self.nc.sync.nop(hint=f"split_drain_{proc}")
            wait_clock.add_sem_waits(nop.ins, ScopedClock({None: vc}))
        self.nc.sync.drain()
        self.nc.all_engine_barrier()
        assert self.sems is not None
        popped = self.nc._tile_sem_poison_stack.pop()
        assert popped is self._sem_poison
        self.nc.clear_and_free_semaphores(list(self.sems.allocated().values()))
        self.nc.all_engine_barrier()


def build_nc(reps: int = 1):
    nc = bacc.Bacc("TRN2", target_bir_lowering=False, num_devices=8)

    xd = nc.declare_dram_parameter("x", [C, L], F32, isOutput=False)
    wqkvT = nc.declare_dram_parameter("wqkvT", [C, 3 * C], BF16, isOutput=False)
    wv8d = nc.declare_dram_parameter("wv8", [128, CT, C], FP8, isOutput=False)
    woutT = nc.declare_dram_parameter("woutT", [C, C], BF16, isOutput=False)
    gnwd = nc.declare_dram_parameter("gnw", [CT, 128], F32, isOutput=False)
    gnbd = nc.declare_dram_parameter("gnb", [CT, 128], F32, isOutput=False)
    boutd = nc.declare_dram_parameter("bout", [128, CT], F32, isOutput=False)
    identd = nc.declare_dram_parameter("ident", [128, 128], F32, isOutput=False)
    yd = nc.declare_dram_parameter("y", [C, LQ], F32, isOutput=True)

    import contextlib

    with _SplitDrainTC(nc) as tc:
        with (
            tc.For_i(0, reps, 1) if reps > 1 else contextlib.nullcontext()
        ), tc.tile_pool(name="persist", bufs=1) as pp:
            x_sb = [pp.tile([128, L], F32, name=f"x{t}", tag=f"x{t}") for t in range(CT)]
            wq_sb = [pp.tile([128, 3 * C], BF16, name=f"wq{t}", tag=f"wq{t}") for t in range(CT)]
            wo_sb = [pp.tile([128, C], BF16, name=f"wo{t}", tag=f"wo{t}") for t in range(CT)]
            nx_sb = [pp.tile([128, L], BF16, name=f"nx{t}", tag=f"nx{t}") for t in range(CT)]
            # fp8 copy of nx (single tensor so DoubleRow APs can span channel
            # tile pairs) + fp8 v-weights: the V^T matmul runs fp8 DoubleRow.
            nx8_sb = pp.tile([128, CT, L], FP8, name="nx8", tag="nx8")
            wv8_sb = pp.tile([128, CT, C], FP8, name="wv8", tag="wv8")
            q_sb = [pp.tile([128, LQ], BF16, name=f"q{t}", tag=f"q{t}") for t in range(CT)]
            k_sb = [pp.tile([128, L], BF16, name=f"k{t}", tag=f"k{t}") for t in range(CT)]
            # V^T double-chunks: [key-in-chunk, ko, head*DP + d]; d=64 is the
            # ones column (denominator), d=65 zero padding.
            vt_sb = [
                pp.tile([128, 2, H, DP], FP8, name=f"vt{j}", tag=f"vt{j}")
                for j in range(NJ2)
            ]
            oh_sb = [pp.tile([128, LQ], F32, name=f"oh{t}", tag=f"oh{t}") for t in range(CT)]
            ohb_sb = [pp.tile([128, LQ], BF16, name=f"ohb{t}", tag=f"ohb{t}") for t in range(CT)]
            gnw_sb = pp.tile([CT, 128], F32, name="gnw", tag="gnw")
            gnb_sb = pp.tile([CT, 128], F32, name="gnb", tag="gnb")
            bout_sb = pp.tile([128, CT], F32, name="bout", tag="bout")
            ones_sb = pp.tile([128, 64], BF16, name="ones", tag="ones")
            nc.vector.memset(ones_sb[:], 1.0)
            ident_sb = pp.tile([128, 128], F32, name="ident", tag="ident")
            sparam_sb = pp.tile([128, 3, CT], F32, name="sparam", tag="sparam")
            ebias_sb = pp.tile([128, 1], F32, name="ebias", tag="ebias")
            nc.vector.memset(ebias_sb[:], EXP_BIAS)
            dummy_sb = pp.tile([1, 1], F32, name="dummy", tag="dummy")
            nc.vector.memset(dummy_sb[:], 1.0)

            # ---------------- input DMA (SP + Pool only; ACT stays clean) ----
            nc.gpsimd.dma_start(ident_sb[:], identd[:])
            nc.gpsimd.dma_start(gnw_sb[:], gnwd[:])
            nc.gpsimd.dma_start(gnb_sb[:], gnbd[:])
            nc.gpsimd.dma_start(bout_sb[:], boutd[:])
            xdma_engs = [nc.sync, nc.gpsimd, nc.scalar]
            di = 0
            for t in range(CT):
                for sg in range(4):
                    csl = slice(512 * sg, 512 * sg + 512)
                    xdma_engs[di % 3].dma_start(
                        x_sb[t][:, csl], xd[128 * t : 128 * t + 128, csl]
                    )
                    di += 1
            dma_engs = [nc.sync, nc.gpsimd]
            for t in range(CT):
                for half in range(2):
                    wsl = slice(768 * half, 768 * half + 768)
                    dma_engs[di % 2].dma_start(
                        wq_sb[t][:, wsl], wqkvT[128 * t : 128 * t + 128, wsl]
                    )
                    di += 1
                dma_engs[di % 2].dma_start(
                    wo_sb[t][:], woutT[128 * t : 128 * t + 128, :]
                )
                di += 1
            nc.sync.dma_start(wv8_sb[:], wv8d[:])
            # dummy Ln pulls the natural_log_exp table load (1.3us) off the
            # stats critical path: it happens here while ACT is otherwise idle
            nc.scalar.activation(out=dummy_sb[:], in_=dummy_sb[:], func=ACTF.Ln)

            # ---------------- group norm statistics ----------------
            with (
                tc.tile_pool(name="gtmp", bufs=2) as gp,
                tc.tile_pool(name="gps", bufs=2, space="PSUM") as gpp,
            ):
                # stats_all col t = channel-mean(tile t), col 32+t = channel-var:
                # after PE transpose, means land on partitions 0..3 and vars on
                # 32..35 (engine APs may only start at partition 0/32/64/96).
                stats_all = gp.tile([128, 36], F32, name="stats_all", tag="stats_all")
                nc.vector.memset(stats_all[:], 0.0)
                for t in range(CT):
                    st6 = gp.tile([128, 4, 6], F32, name="st6", tag="st6")
                    for sg in range(4):
                        nc.vector.bn_stats(
                            out=st6[:, sg, :],
                            in_=x_sb[t][:, 512 * sg : 512 * sg + 512],
                        )
                    sa = stats_all[:]
                    mv_out = bass.AP(
                        tensor=sa.tensor, offset=sa.offset + t, ap=[sa.ap[0], [32, 2]]
                    )
                    nc.vector.bn_aggr(out=mv_out, in_=st6[:])

                st_ps = gpp.tile([36, 128], F32, name="st_ps", tag="st_ps")
                nc.tensor.transpose(st_ps[:], stats_all[:], ident_sb[:])
                statsT = gp.tile([36, 128], F32, name="statsT", tag="statsT")
                nc.vector.tensor_copy(statsT[:], st_ps[:])

                mred = gp.tile([4, 8], F32, name="mred", tag="mred")
                nc.vector.tensor_reduce(
                    out=mred[:],
                    in_=statsT[0:4, :].rearrange("p (g s) -> p g s", s=16),
                    axis=AX.X,
                    op=OP.add,
                )
                vred = gp.tile([4, 8], F32, name="vred", tag="vred")
                nc.vector.tensor_reduce(
                    out=vred[:],
                    in_=statsT[32:36, :].rearrange("p (g s) -> p g s", s=16),
                    axis=AX.X,
                    op=OP.add,
                )
                sq = gp.tile([4, 128], F32, name="sq", tag="sq")
                nc.vector.tensor_mul(sq[:], statsT[0:4, :], statsT[0:4, :])
                sqred = gp.tile([4, 8], F32, name="sqred", tag="sqred")
                nc.vector.tensor_reduce(
                    out=sqred[:],
                    in_=sq[:].rearrange("p (g s) -> p g s", s=16),
                    axis=AX.X,
                    op=OP.add,
                )
                mg = gp.tile([4, 8], F32, name="mg", tag="mg")
                nc.vector.tensor_scalar_mul(mg[:], mred[:], 1.0 / 16)
                # vg = red_var/16 + sqred/16 - mg^2
                vg = gp.tile([4, 8], F32, name="vg", tag="vg")
                nc.vector.tensor_scalar_mul(vg[:], vred[:], 1.0 / 16)
                nc.vector.scalar_tensor_tensor(
                    out=vg[:],
                    in0=sqred[:],
                    scalar=1.0 / 16,
                    in1=vg[:],
                    op0=OP.mult,
                    op1=OP.add,
                )
                mg2 = gp.tile([4, 8], F32, name="mg2", tag="mg2")
                nc.vector.tensor_mul(mg2[:], mg[:], mg[:])
                nc.vector.tensor_sub(vg[:], vg[:], mg2[:])
                # rstd = (vg + eps)^-0.5 = exp(-0.5*ln(vg + eps)); Log and Exp
                # share one ACT table set with the softmax exp, so the kernel
                # pays exactly one table load.
                epst = gp.tile([4, 1], F32, name="epst", tag="epst")
                nc.vector.memset(epst[:], EPS)
                lvg = gp.tile([4, 8], F32, name="lvg", tag="lvg")
                nc.scalar.activation(out=lvg[:], in_=vg[:], func=ACTF.Ln, bias=epst[:])
                nc.scalar.activation(out=vg[:], in_=lvg[:], func=ACTF.Exp, scale=-0.5)

                # broadcast group -> channels: [4, 8] -> [4, 128]
                def bcast16(src):
                    a = src.ap
                    return bass.AP(
                        tensor=src.tensor, offset=src.offset, ap=[a[0], a[1], [0, 16]]
                    )

                rstd_bc = gp.tile([4, 128], F32, name="rstd_bc", tag="rstd_bc")
                nc.vector.tensor_copy(
                    rstd_bc[:].rearrange("p (g s) -> p g s", s=16), bcast16(vg[:])
                )
                mg_bc = gp.tile([4, 128], F32, name="mg_bc", tag="mg_bc")
                nc.vector.tensor_copy(
                    mg_bc[:].rearrange("p (g s) -> p g s", s=16), bcast16(mg[:])
                )
                s2 = gp.tile([4, 128], F32, name="s2", tag="s2")
                nc.vector.tensor_mul(s2[:], rstd_bc[:], gnw_sb[0:4, :])
                s1 = gp.tile([4, 128], F32, name="s1", tag="s1")
                nc.vector.reciprocal(out=s1[:], in_=s2[:])
                nc.vector.tensor_mul(s1[:], s1[:], gnb_sb[0:4, :])
                nc.vector.tensor_sub(s1[:], mg_bc[:], s1[:])

                # third column: -(s1*s2), the bias form ACT's activation needs
                # for nx = x*s2 + (-s1*s2)
                s12 = gp.tile([4, 128], F32, name="s12", tag="s12")
                nc.vector.scalar_tensor_tensor(
                    out=s12[:], in0=s1[:], scalar=-1.0, in1=s2[:],
                    op0=OP.mult, op1=OP.mult,
                )
                sp_ps = gpp.tile([128, 3, CT], F32, name="sp_ps", tag="sp_ps")
                nc.tensor.transpose(sp_ps[:, 0, :], s1[:], ident_sb[0:4, 0:4])
                nc.tensor.transpose(sp_ps[:, 1, :], s2[:], ident_sb[0:4, 0:4])
                nc.tensor.transpose(sp_ps[:, 2, :], s12[:], ident_sb[0:4, 0:4])
                nc.vector.tensor_copy(sparam_sb[:], sp_ps[:])

            # group-norm apply: nx = (x - s1) * s2, cast to bf16 (+ fp8 for V).
            # Tile 3 goes through ACT (idle here) so the DVE finishes sooner.
            for t in range(3):
                nc.vector.tensor_scalar(
                    out=nx_sb[t][:],
                    in0=x_sb[t][:],
                    scalar1=sparam_sb[:, 0, t : t + 1],
                    scalar2=sparam_sb[:, 1, t : t + 1],
                    op0=OP.subtract,
                    op1=OP.mult,
                )
            nc.scalar.activation(
                out=nx_sb[3][:],
                in_=x_sb[3][:],
                func=ACTF.Identity,
                scale=sparam_sb[:, 1, 3:4],
                bias=sparam_sb[:, 2, 3:4],
            )
            def emit_nx8():
                with nc.allow_low_precision(reason="fp8 V-path intended"):
                    for t in range(CT):
                        nc.vector.tensor_scalar(
                            out=nx8_sb[:, t, :],
                            in0=x_sb[t][:],
                            scalar1=sparam_sb[:, 0, t : t + 1],
                            scalar2=sparam_sb[:, 1, t : t + 1],
                            op0=OP.subtract,
                            op1=OP.mult,
                        )

            # ---------------- attention + interleaved qkv/proj ----------------
            with (
                tc.tile_pool(name="psS", bufs=2, space="PSUM") as pS,
                tc.tile_pool(name="psO", bufs=1, space="PSUM") as pO,
                tc.tile_pool(name="psW", bufs=2, space="PSUM") as pW,
                tc.tile_pool(name="expp", bufs=3) as ep,
                tc.tile_pool(name="rcpp", bufs=2) as rp,
            ):
                # Spare work is sliced into single-matmul units so the PE
                # stream between attention chunks never bursts long enough to
                # delay the next S^T matmul (which gates the exp stream).
                def chain_units(tag, mm_args, finish, n=CT, perf_mode=None):
                    cell = {}
                    units = []

                    def mk(c):
                        def u():
                            if c == 0:
                                cell["ps"] = pW.tile([128, 512], F32, name="w", tag="w")
                            lhsT, rhs = mm_args(c)
                            nc.tensor.matmul(
                                cell["ps"][:], lhsT, rhs,
                                start=(c == 0), stop=(c == n - 1),
                                perf_mode=perf_mode,
                            )
                            if c == n - 1:
                                finish(cell["ps"])
                        return u

                    return [(tag, mk(c)) for c in range(n)]

                def k_units(t, nb):
                    def fin(ps):
                        nc.vector.tensor_copy(
                            k_sb[t][:, 512 * nb : 512 * nb + 512], ps[:]
                        )
                    return chain_units(
                        ("k", t, nb),
                        lambda c: (
                            wq_sb[c][:, C + 128 * t : C + 128 * t + 128],
                            nx_sb[c][:, 512 * nb : 512 * nb + 512],
                        ),
                        fin,
                    )

                def q_units(t, nb):
                    def fin(ps):
                        nc.vector.tensor_copy(
                            q_sb[t][:, 512 * nb : 512 * nb + 512], ps[:]
                        )
                    return chain_units(
                        ("q", t, nb),
                        lambda c: (
                            wq_sb[c][:, 128 * t : 128 * t + 128],
                            nx_sb[c][:, 512 * nb : 512 * nb + 512],
                        ),
                        fin,
                    )

                def vt_units(j2, ko):
                    lt = 2 * j2 + ko

                    def fin(ps):
                        with nc.allow_low_precision(reason="fp8 attention intended"):
                            if ko == 0:
                                nc.vector.memset(vt_sb[j2][:, :, :, D : D + 1], 1.0)
                                nc.vector.memset(vt_sb[j2][:, :, :, D + 1 : D + 2], 0.0)
                            nc.vector.tensor_copy(
                                vt_sb[j2][:, ko, :, 0:D],
                                ps[:].rearrange("p (h d) -> p h d", d=D),
                            )

                    # fp8 DoubleRow over channel-tile pairs: 2 MMs instead of 4
                    return chain_units(
                        ("vt", j2),
                        lambda u: (
                            nx8_sb[:, 2 * u : 2 * u + 2, 128 * lt : 128 * lt + 128],
                            wv8_sb[:, 2 * u : 2 * u + 2, :],
                        ),
                        fin,
                        n=2,
                        perf_mode=PMODE.DoubleRow,
                    )

                def proj_units(t, ib):
                    sl = slice(512 * ib, 512 * ib + 512)

                    def fin(ps):
                        # y = (proj + b_out) + residual, fused; reuse oh as staging
                        nc.vector.scalar_tensor_tensor(
                            out=oh_sb[t][:, sl],
                            in0=ps[:],
                            scalar=bout_sb[:, t : t + 1],
                            in1=x_sb[t][:, sl],
                            op0=OP.add,
                            op1=OP.add,
                        )
                        nc.sync.dma_start(
                            yd[128 * t : 128 * t + 128, sl], oh_sb[t][:, sl]
                        )

                    return chain_units(
                        ("proj", ib),
                        lambda c: (
                            wo_sb[c][:, 128 * t : 128 * t + 128],
                            ohb_sb[c][:, sl],
                        ),
                        fin,
                    )

                # prefix: just enough for (ib0, pair0, jc0) to start; the fp8
                # nx copy comes after so its DVE work doesn't delay the first
                # k/q PSUM->SBUF copies (and with them the first exp).
                for _, u in k_units(0, 0):
                    u()
                for _, u in q_units(0, 0):
                    u()
                emit_nx8()

                # spare-work queue (ordered; consumed between attention chunks;
                # `ensure` pulls a specific chain's remaining units just-in-time)
                work = []
                # interleave the k0 chunks among the vt chunks so each is a
                # little ahead of its just-in-time `ensure` point in pair 0
                work += vt_units(0, 0) + vt_units(0, 1)
                work += vt_units(1, 0) + vt_units(1, 1) + k_units(0, 1)
                work += vt_units(2, 0) + vt_units(2, 1)
                work += vt_units(3, 0) + vt_units(3, 1) + k_units(0, 2)
                work += vt_units(4, 0) + vt_units(4, 1)
                work += vt_units(5, 0) + vt_units(5, 1) + k_units(0, 3)
                work += vt_units(6, 0) + vt_units(6, 1)
                work += vt_units(7, 0) + vt_units(7, 1)
                # remaining chunks in the order the attention loop needs them
                for t in range(1, CT):
                    work += q_units(t, 0) + k_units(t, 0)
                    work += k_units(t, 1) + k_units(t, 2) + k_units(t, 3)
                for t in range(CT):
                    work += q_units(t, 1)

                def emit_spare(n):
                    for _ in range(n):
                        if work:
                            work.pop(0)[1]()

                def ensure(tag):
                    i = 0
                    while i < len(work):
                        if work[i][0] == tag:
                            work.pop(i)[1]()
                        else:
                            i += 1

                for ib in range(NIB):
                    qsl = slice(512 * ib, 512 * ib + 512)
                    if ib == NIB - 1:
                        for t in range(CT):
                            ensure(("q", t, ib))
                    for pair in range(H // 2):
                        ensure(("q", pair, ib))
                        kt, qt = k_sb[pair], q_sb[pair]
                        ops = pO.tile([128, 2, 512], F32, name="O", tag="O")
                        for j2 in range(NJ2):
                            et = ep.tile([128, 2, 1024], FP8, name="exp", tag="exp")
                            for ko in range(2):
                                jc = 2 * j2 + ko
                                ensure(("k", pair, jc // 4))
                                slots = pS.tile([128, 2, 512], F32, name="S", tag="S")
                                nc.tensor.matmul(
                                    slots[:, 0, :],
                                    kt[0:64, 128 * jc : 128 * jc + 128],
                                    qt[0:64, qsl],
                                    start=True,
                                    stop=True,
                                )
                                nc.tensor.matmul(
                                    slots[:, 1, :],
                                    kt[64:128, 128 * jc : 128 * jc + 128],
                                    qt[64:128, qsl],
                                    start=True,
                                    stop=True,
                                )
                                with nc.allow_low_precision(reason="fp8 softmax intended"):
                                    nc.scalar.activation(
                                        out=et[:, ko, :],
                                        in_=slots[:].rearrange("p a b -> p (a b)"),
                                        func=ACTF.Exp,
                                        scale=float(D) ** -0.5,
                                        bias=ebias_sb[:],
                                    )
                                emit_spare(3 if len(work) > 80 else 2)
                            ensure(("vt", j2))
                            for h01 in range(2):
                                nc.tensor.matmul(
                                    ops[0 : D + 2, h01, :],
                                    vt_sb[j2][:, :, 2 * pair + h01, :],
                                    et[:].rearrange("p a (h n) -> p a h n", n=512)[
                                        :, :, h01, :
                                    ],
                                    start=(j2 == 0),
                                    stop=(j2 == NJ2 - 1),
                                    perf_mode=PMODE.DoubleRow,
                                )
                            emit_spare(1)
                        # ---- pair done: copy O out, normalize in place ----
                        # softmax denominators sit on PSUM partition 64 (ones
                        # column of V^T); reciprocal them there, then a K=1
                        # matmul with a ones row at partition 64 broadcasts
                        # 1/denom down to the head's 64 output channels.
                        rcps = rp.tile([128, 2, 512], BF16, name="rcps", tag="rcps")
                        with nc.allow_low_precision(reason="bf16 softmax recip"):
                            nc.vector.reciprocal(
                                out=rcps[64:65, :, :], in_=ops[64:65, :, :]
                            )
                        for h01 in range(2):
                            nc.vector.tensor_copy(
                                oh_sb[pair][64 * h01 : 64 * h01 + 64, qsl],
                                ops[0:64, h01, :],
                            )
                        bps = pW.tile([128, 512], F32, name="w", tag="w")
                        for h01 in range(2):
                            nc.tensor.matmul(
                                bps[64 * h01 : 64 * h01 + 64, :],
                                ones_sb[64:65, 0:64],
                                rcps[64:65, h01, :],
                                start=True,
                                stop=True,
                            )
                        nc.vector.tensor_mul(
                            ohb_sb[pair][:, qsl], oh_sb[pair][:, qsl], bps[:]
                        )
                    # ---- ib done: queue (or emit) its projection ----
                    if ib < NIB - 1:
                        for t in range(CT):
                            work += proj_units(t, ib)
                    else:
                        emit_spare(len(work))  # drain any stragglers
                        for t in range(CT):
                            for _, u in proj_units(t, ib):
                                u()

    nc.compile()
    return nc


_NC_CACHE = None


def _get_nc():
    global _NC_CACHE
    if _NC_CACHE is None:
        _NC_CACHE = build_nc()
    return _NC_CACHE


def _host_inputs(x, gn_w, gn_b, w_qkv, w_out, b_out):
    w_qkvT = np.ascontiguousarray(w_qkv.T).astype(ml_dtypes.bfloat16)
    w_outT = np.ascontiguousarray(w_out.T).astype(ml_dtypes.bfloat16)
    wv8 = np.ascontiguousarray(
        np.asarray(w_qkv[2 * C : 3 * C, :].T, np.float32)
        .reshape(CT, 128, C)
        .transpose(1, 0, 2)
    ).astype(ml_dtypes.float8_e4m3fn)
    ident = np.eye(128, dtype=np.float32)
    shared = {
        "wqkvT": w_qkvT,
        "wv8": wv8,
        "woutT": w_outT,
        "gnw": np.ascontiguousarray(gn_w.reshape(CT, 128), np.float32),
        "gnb": np.ascontiguousarray(gn_b.reshape(CT, 128), np.float32),
        "bout": np.ascontiguousarray(b_out.reshape(CT, 128).T, np.float32),
        "ident": ident,
    }
    in_maps = []
    for core in range(8):
        b, ih = core // 2, core % 2
        xb = np.asarray(x[b], np.float32)
        if ih:
            xb = np.concatenate([xb[:, LQ:], xb[:, :LQ]], axis=1)
        in_maps.append({"x": np.ascontiguousarray(xb), **shared})
    return in_maps


def kernel(x, gn_w, gn_b, w_qkv, w_out, b_out):
    nc = _get_nc()
    in_maps = _host_inputs(
        np.asarray(x), np.asarray(gn_w), np.asarray(gn_b),
        np.asarray(w_qkv), np.asarray(w_out), np.asarray(b_out),
    )
    res = run_bass_kernel_spmd(nc, in_maps, list(range(8)))
    y = np.empty((B, C, L), np.float32)
    for core in range(8):
        b, ih = core // 2, core % 2
        y[b][:, ih * LQ : (ih + 1) * LQ] = res.results[core]["y"]
    return y


# revision 47
# speedup vs baseline: 1.7703x; 1.0108x over previous
"""AttentionBlock (GroupNorm + 8-head self-attention + out-proj + residual) on 8 trn2 cores.

Sharding: core = (batch b, query-half ih).  Each core gets x[b] rolled so that
"its" 1024 query positions are columns 0:1024; K/V are computed over the full
(rolled) L=2048, which is sound because attention and the group-norm statistics
are invariant to a permutation of key/value positions.  Output is the core's
[512, 1024] slice of proj + residual; the host reassembles [4, 512, 2048].

Structure (v2, ACT-saturating):
 - The softmax exp is the hard floor (~133us of ACT time per core: 16.8M
   elements at 1 elem/cycle/lane @1.2GHz + per-inst bubble).  Everything else
   is organized to hide under a continuous stream of 128 exp instructions.
 - exp reads S^T straight from PSUM ([128 keys, 2x512 cols] per 128-key chunk)
   and writes fp8e4m3 to SBUF with the 1/8 scale and a -3 logit bias folded in
   (softmax is shift-invariant; keeps exp < 448 = fp8 max).
 - O = V^T-augmented @ exp runs in fp8 with perf_mode=DoubleRow: 256 keys per
   MM via the [128, 2, *] k-interleave, half the PE time of bf16.  A ones
   column in V^T produces the softmax denominator on PSUM partition 64.
 - The V^T = nx^T Wv matmuls also run fp8 DoubleRow (256 channels per MM) off
   a dedicated fp8 copy of nx, so pair 0 can feed V^T just-in-time without
   starving the exp stream.
 - All other PE work (remaining qkv tiles, V^T chunks, out-proj) is emitted as
   single-matmul "spare work" units interleaved between attention chunks (a
   tagged queue with just-in-time `ensure` pulls), so the PE never bursts long
   enough to starve ACT.
 - Softmax division: 1/denominator is computed in place on PSUM partition 64,
   then a K=1 matmul against a ones row at partition 64 broadcasts it to the
   head's 64 channels - no cross-partition DMA, normalize is per-pair local.
 - ACT does nothing else: group-norm rstd uses Ln+Exp (same ACT table set as
   the softmax exp; a dummy Ln at kernel start hoists the table load off the
   critical path), one nx tile is applied via ACT activation(Identity) to
   shorten the DVE prefix, and bulk DMAs go on SP/Pool queues.
 - PSUM: 4 banks S^T (2 rotations), 2 banks O accum, 2 banks qkv/proj chains.
"""

import sys

sys.path.insert(0, "/opt/trn_rl_repo")

import numpy as np
import ml_dtypes

import concourse.bass as bass
import concourse.mybir as mybir
import concourse.tile as tile
from concourse import bacc
from concourse.vector_clock import ScopedClock, VectorClock
from concourse.bass_utils import run_bass_kernel_spmd

F32 = mybir.dt.float32
BF16 = mybir.dt.bfloat16
FP8 = mybir.dt.float8e4
AX = mybir.AxisListType
OP = mybir.AluOpType
ACTF = mybir.ActivationFunctionType
PMODE = mybir.MatmulPerfMode

B, C, L = 4, 512, 2048
H, D = 8, 64
G, EPS = 32, 1e-5
LQ = L // 2          # queries per core
CT = C // 128        # channel tiles
NJC = L // 128       # key chunks of 128
NJ2 = NJC // 2       # key double-chunks of 256 (DoubleRow granularity)
NIB = LQ // 512      # 512-wide query blocks
DP = D + 2           # V^T head stride: 64 V cols + ones col + zero pad (66, so 8*66 % 16 == 0)
EXP_BIAS = -3.0      # exp(s/8 - 3): softmax-invariant shift, keeps fp8 in range


class _SplitDrainTC(tile.TileContext):
    """Stock exit puts every outstanding proc's wait on one SP Drain; this
    walrus build caps sync-waits per instruction, so spread them over
    single-wait NOPs first."""

    def _drain_and_barrier(self, tick_clock, wait_clock):
        g = tick_clock.global_clock
        for proc in range(len(g)):
            if g[proc] == 0:
                continue
            vc = VectorClock([0] * len(g))
            vc.require_at_least(proc, g[proc])
            nop = self.nc.sync.nop(hint=f"split_drain_{proc}")
            wait_clock.add_sem_waits(nop.ins, ScopedClock({None: vc}))
        self.nc.sync.drain()
        self.nc.all_engine_barrier()
        assert self.sems is not None
        popped = self.nc._tile_sem_poison_stack.pop()
        assert popped is self._sem_poison
        self.nc.clear_and_free_semaphores(list(self.sems.allocated().values()))
        self.nc.all_engine_barrier()


def build_nc(reps: int = 1):
    nc = bacc.Bacc("TRN2", target_bir_lowering=False, num_devices=8)

    xd = nc.declare_dram_parameter("x", [C, L], F32, isOutput=False)
    wqkvT = nc.declare_dram_parameter("wqkvT", [C, 3 * C], BF16, isOutput=False)
    wv8d = nc.declare_dram_parameter("wv8", [128, CT, C], FP8, isOutput=False)
    woutT = nc.declare_dram_parameter("woutT", [C, C], BF16, isOutput=False)
    gnwd = nc.declare_dram_parameter("gnw", [CT, 128], F32, isOutput=False)
    gnbd = nc.declare_dram_parameter("gnb", [CT, 128], F32, isOutput=False)
    boutd = nc.declare_dram_parameter("bout", [128, CT], F32, isOutput=False)
    identd = nc.declare_dram_parameter("ident", [128, 128], F32, isOutput=False)
    yd = nc.declare_dram_parameter("y", [C, LQ], F32, isOutput=True)

    import contextlib

    with _SplitDrainTC(nc) as tc:
        with (
            tc.For_i(0, reps, 1) if reps > 1 else contextlib.nullcontext()
        ), tc.tile_pool(name="persist", bufs=1) as pp:
            x_sb = [pp.tile([128, L], F32, name=f"x{t}", tag=f"x{t}") for t in range(CT)]
            wq_sb = [pp.tile([128, 3 * C], BF16, name=f"wq{t}", tag=f"wq{t}") for t in range(CT)]
            wo_sb = [pp.tile([128, C], BF16, name=f"wo{t}", tag=f"wo{t}") for t in range(CT)]
            nx_sb = [pp.tile([128, L], BF16, name=f"nx{t}", tag=f"nx{t}") for t in range(CT)]
            # fp8 copy of nx (single tensor so DoubleRow APs can span channel
            # tile pairs) + fp8 v-weights: the V^T matmul runs fp8 DoubleRow.
            nx8_sb = pp.tile([128, CT, L], FP8, name="nx8", tag="nx8")
            wv8_sb = pp.tile([128, CT, C], FP8, name="wv8", tag="wv8")
            q_sb = [pp.tile([128, LQ], BF16, name=f"q{t}", tag=f"q{t}") for t in range(CT)]
            k_sb = [pp.tile([128, L], BF16, name=f"k{t}", tag=f"k{t}") for t in range(CT)]
            # V^T double-chunks: [key-in-chunk, ko, head*DP + d]; d=64 is the
            # ones column (denominator), d=65 zero padding.
            vt_sb = [
                pp.tile([128, 2, H, DP], FP8, name=f"vt{j}", tag=f"vt{j}")
                for j in range(NJ2)
            ]
            oh_sb = [pp.tile([128, LQ], F32, name=f"oh{t}", tag=f"oh{t}") for t in range(CT)]
            ohb_sb = [pp.tile([128, LQ], BF16, name=f"ohb{t}", tag=f"ohb{t}") for t in range(CT)]
            gnw_sb = pp.tile([CT, 128], F32, name="gnw", tag="gnw")
            gnb_sb = pp.tile([CT, 128], F32, name="gnb", tag="gnb")
            bout_sb = pp.tile([128, CT], F32, name="bout", tag="bout")
            ones_sb = pp.tile([128, 64], BF16, name="ones", tag="ones")
            nc.vector.memset(ones_sb[:], 1.0)
            ident_sb = pp.tile([128, 128], F32, name="ident", tag="ident")
            sparam_sb = pp.tile([128, 3, CT], F32, name="sparam", tag="sparam")
            ebias_sb = pp.tile([128, 1], F32, name="ebias", tag="ebias")
            nc.vector.memset(ebias_sb[:], EXP_BIAS)
            dummy_sb = pp.tile([1, 1], F32, name="dummy", tag="dummy")
            nc.vector.memset(dummy_sb[:], 1.0)

            # ---------------- input DMA (SP + Pool only; ACT stays clean) ----
            nc.gpsimd.dma_start(ident_sb[:], identd[:])
            nc.gpsimd.dma_start(gnw_sb[:], gnwd[:])
            nc.gpsimd.dma_start(gnb_sb[:], gnbd[:])
            nc.gpsimd.dma_start(bout_sb[:], boutd[:])
            xdma_engs = [nc.sync, nc.gpsimd, nc.scalar]
            di = 0
            for t in range(CT):
                for sg in range(4):
                    csl = slice(512 * sg, 512 * sg + 512)
                    xdma_engs[di % 3].dma_start(
                        x_sb[t][:, csl], xd[128 * t : 128 * t + 128, csl]
                    )
                    di += 1
            dma_engs = [nc.sync, nc.gpsimd]
            for t in range(CT):
                for half in range(2):
                    wsl = slice(768 * half, 768 * half + 768)
                    dma_engs[di % 2].dma_start(
                        wq_sb[t][:, wsl], wqkvT[128 * t : 128 * t + 128, wsl]
                    )
                    di += 1
                dma_engs[di % 2].dma_start(
                    wo_sb[t][:], woutT[128 * t : 128 * t + 128, :]
                )
                di += 1
            nc.sync.dma_start(wv8_sb[:], wv8d[:])
            # dummy Ln pulls the natural_log_exp table load (1.3us) off the
            # stats critical path: it happens here while ACT is otherwise idle
            nc.scalar.activation(out=dummy_sb[:], in_=dummy_sb[:], func=ACTF.Ln)

            # ---------------- group norm statistics ----------------
            with (
                tc.tile_pool(name="gtmp", bufs=2) as gp,
                tc.tile_pool(name="gps", bufs=2, space="PSUM") as gpp,
            ):
                # stats_all col t = channel-mean(tile t), col 32+t = channel-var:
                # after PE transpose, means land on partitions 0..3 and vars on
                # 32..35 (engine APs may only start at partition 0/32/64/96).
                stats_all = gp.tile([128, 36], F32, name="stats_all", tag="stats_all")
                nc.vector.memset(stats_all[:], 0.0)
                for t in range(CT):
                    st6 = gp.tile([128, 4, 6], F32, name="st6", tag="st6")
                    for sg in range(4):
                        nc.vector.bn_stats(
                            out=st6[:, sg, :],
                            in_=x_sb[t][:, 512 * sg : 512 * sg + 512],
                        )
                    sa = stats_all[:]
                    mv_out = bass.AP(
                        tensor=sa.tensor, offset=sa.offset + t, ap=[sa.ap[0], [32, 2]]
                    )
                    nc.vector.bn_aggr(out=mv_out, in_=st6[:])

                st_ps = gpp.tile([36, 128], F32, name="st_ps", tag="st_ps")
                nc.tensor.transpose(st_ps[:], stats_all[:], ident_sb[:])
                statsT = gp.tile([36, 128], F32, name="statsT", tag="statsT")
                nc.vector.tensor_copy(statsT[:], st_ps[:])

                mred = gp.tile([4, 8], F32, name="mred", tag="mred")
                nc.vector.tensor_reduce(
                    out=mred[:],
                    in_=statsT[0:4, :].rearrange("p (g s) -> p g s", s=16),
                    axis=AX.X,
                    op=OP.add,
                )
                vred = gp.tile([4, 8], F32, name="vred", tag="vred")
                nc.vector.tensor_reduce(
                    out=vred[:],
                    in_=statsT[32:36, :].rearrange("p (g s) -> p g s", s=16),
                    axis=AX.X,
                    op=OP.add,
                )
                sq = gp.tile([4, 128], F32, name="sq", tag="sq")
                nc.vector.tensor_mul(sq[:], statsT[0:4, :], statsT[0:4, :])
                sqred = gp.tile([4, 8], F32, name="sqred", tag="sqred")
                nc.vector.tensor_reduce(
                    out=sqred[:],
                    in_=sq[:].rearrange("p (g s) -> p g s", s=16),
                    axis=AX.X,
                    op=OP.add,
                )
                mg = gp.tile([4, 8], F32, name="mg", tag="mg")
                nc.vector.tensor_scalar_mul(mg[:], mred[:], 1.0 / 16)
                # vg = red_var/16 + sqred/16 - mg^2
                vg = gp.tile([4, 8], F32, name="vg", tag="vg")
                nc.vector.tensor_scalar_mul(vg[:], vred[:], 1.0 / 16)
                nc.vector.scalar_tensor_tensor(
                    out=vg[:],
                    in0=sqred[:],
                    scalar=1.0 / 16,
                    in1=vg[:],
                    op0=OP.mult,
                    op1=OP.add,
                )
                mg2 = gp.tile([4, 8], F32, name="mg2", tag="mg2")
                nc.vector.tensor_mul(mg2[:], mg[:], mg[:])
                nc.vector.tensor_sub(vg[:], vg[:], mg2[:])
                # rstd = (vg + eps)^-0.5 = exp(-0.5*ln(vg + eps)); Log and Exp
                # share one ACT table set with the softmax exp, so the kernel
                # pays exactly one table load.
                epst = gp.tile([4, 1], F32, name="epst", tag="epst")
                nc.vector.memset(epst[:], EPS)
                lvg = gp.tile([4, 8], F32, name="lvg", tag="lvg")
                nc.scalar.activation(out=lvg[:], in_=vg[:], func=ACTF.Ln, bias=epst[:])
                nc.scalar.activation(out=vg[:], in_=lvg[:], func=ACTF.Exp, scale=-0.5)

                # broadcast group -> channels: [4, 8] -> [4, 128]
                def bcast16(src):
                    a = src.ap
                    return bass.AP(
                        tensor=src.tensor, offset=src.offset, ap=[a[0], a[1], [0, 16]]
                    )

                rstd_bc = gp.tile([4, 128], F32, name="rstd_bc", tag="rstd_bc")
                nc.vector.tensor_copy(
                    rstd_bc[:].rearrange("p (g s) -> p g s", s=16), bcast16(vg[:])
                )
                mg_bc = gp.tile([4, 128], F32, name="mg_bc", tag="mg_bc")
                nc.vector.tensor_copy(
                    mg_bc[:].rearrange("p (g s) -> p g s", s=16), bcast16(mg[:])
                )
                s2 = gp.tile([4, 128], F32, name="s2", tag="s2")
                nc.vector.tensor_mul(s2[:], rstd_bc[:], gnw_sb[0:4, :])
                s1 = gp.tile([4, 128], F32, name="s1", tag="s1")
                nc.vector.reciprocal(out=s1[:], in_=s2[:])
                nc.vector.tensor_mul(s1[:], s1[:], gnb_sb[0:4, :])
                nc.vector.tensor_sub(s1[:], mg_bc[:], s1[:])

                # third column: -(s1*s2), the bias form ACT's activation needs
                # for nx = x*s2 + (-s1*s2)
                s12 = gp.tile([4, 128], F32, name="s12", tag="s12")
                nc.vector.scalar_tensor_tensor(
                    out=s12[:], in0=s1[:], scalar=-1.0, in1=s2[:],
                    op0=OP.mult, op1=OP.mult,
                )
                sp_ps = gpp.tile([128, 3, CT], F32, name="sp_ps", tag="sp_ps")
                nc.tensor.transpose(sp_ps[:, 0, :], s1[:], ident_sb[0:4, 0:4])
                nc.tensor.transpose(sp_ps[:, 1, :], s2[:], ident_sb[0:4, 0:4])
                nc.tensor.transpose(sp_ps[:, 2, :], s12[:], ident_sb[0:4, 0:4])
                nc.vector.tensor_copy(sparam_sb[:], sp_ps[:])

            # group-norm apply: nx = (x - s1) * s2, cast to bf16 (+ fp8 for V).
            # Tile 3 goes through ACT (idle here) so the DVE finishes sooner.
            for t in range(3):
                nc.vector.tensor_scalar(
                    out=nx_sb[t][:],
                    in0=x_sb[t][:],
                    scalar1=sparam_sb[:, 0, t : t + 1],
                    scalar2=sparam_sb[:, 1, t : t + 1],
                    op0=OP.subtract,
                    op1=OP.mult,
                )
            nc.scalar.activation(
                out=nx_sb[3][:],
                in_=x_sb[3][:],
                func=ACTF.Identity,
                scale=sparam_sb[:, 1, 3:4],
                bias=sparam_sb[:, 2, 3:4],
            )
            def emit_nx8():
                with nc.allow_low_precision(reason="fp8 V-path intended"):
                    for t in range(CT):
                        nc.vector.tensor_scalar(
                            out=nx8_sb[:, t, :],
                            in0=x_sb[t][:],
                            scalar1=sparam_sb[:, 0, t : t + 1],
                            scalar2=sparam_sb[:, 1, t : t + 1],
                            op0=OP.subtract,
                            op1=OP.mult,
                        )

            # ---------------- attention + interleaved qkv/proj ----------------
            with (
                tc.tile_pool(name="psS", bufs=2, space="PSUM") as pS,
                tc.tile_pool(name="psO", bufs=1, space="PSUM") as pO,
                tc.tile_pool(name="psW", bufs=2, space="PSUM") as pW,
                tc.tile_pool(name="expp", bufs=3) as ep,
                tc.tile_pool(name="rcpp", bufs=2) as rp,
            ):
                # Spare work is sliced into single-matmul units so the PE
                # stream between attention chunks never bursts long enough to
                # delay the next S^T matmul (which gates the exp stream).
                def chain_units(tag, mm_args, finish, n=CT, perf_mode=None):
                    cell = {}
                    units = []

                    def mk(c):
                        def u():
                            if c == 0:
                                cell["ps"] = pW.tile([128, 512], F32, name="w", tag="w")
                            lhsT, rhs = mm_args(c)
                            nc.tensor.matmul(
                                cell["ps"][:], lhsT, rhs,
                                start=(c == 0), stop=(c == n - 1),
                                perf_mode=perf_mode,
                            )
                            if c == n - 1:
                                finish(cell["ps"])
                        return u

                    return [(tag, mk(c)) for c in range(n)]

                def k_units(t, nb):
                    def fin(ps):
                        nc.vector.tensor_copy(
                            k_sb[t][:, 512 * nb : 512 * nb + 512], ps[:]
                        )
                    return chain_units(
                        ("k", t, nb),
                        lambda c: (
                            wq_sb[c][:, C + 128 * t : C + 128 * t + 128],
                            nx_sb[c][:, 512 * nb : 512 * nb + 512],
                        ),
                        fin,
                    )

                def q_units(t, nb):
                    def fin(ps):
                        nc.vector.tensor_copy(
                            q_sb[t][:, 512 * nb : 512 * nb + 512], ps[:]
                        )
                    return chain_units(
                        ("q", t, nb),
                        lambda c: (
                            wq_sb[c][:, 128 * t : 128 * t + 128],
                            nx_sb[c][:, 512 * nb : 512 * nb + 512],
                        ),
                        fin,
                    )

                def vt_units(j2, ko):
                    lt = 2 * j2 + ko

                    def fin(ps):
                        with nc.allow_low_precision(reason="fp8 attention intended"):
                            if ko == 0:
                                nc.vector.memset(vt_sb[j2][:, :, :, D : D + 1], 1.0)
                                nc.vector.memset(vt_sb[j2][:, :, :, D + 1 : D + 2], 0.0)
                            nc.vector.tensor_copy(
                                vt_sb[j2][:, ko, :, 0:D],
                                ps[:].rearrange("p (h d) -> p h d", d=D),
                            )

                    # fp8 DoubleRow over channel-tile pairs: 2 MMs instead of 4
                    return chain_units(
                        ("vt", j2),
                        lambda u: (
                            nx8_sb[:, 2 * u : 2 * u + 2, 128 * lt : 128 * lt + 128],
                            wv8_sb[:, 2 * u : 2 * u + 2, :],
                        ),
                        fin,
                        n=2,
                        perf_mode=PMODE.DoubleRow,
                    )

                def proj_units(t, ib):
                    sl = slice(512 * ib, 512 * ib + 512)

                    def fin(ps):
                        # y = (proj + b_out) + residual, fused; reuse oh as staging
                        nc.vector.scalar_tensor_tensor(
                            out=oh_sb[t][:, sl],
                            in0=ps[:],
                            scalar=bout_sb[:, t : t + 1],
                            in1=x_sb[t][:, sl],
                            op0=OP.add,
                            op1=OP.add,
                        )
                        nc.sync.dma_start(
                            yd[128 * t : 128 * t + 128, sl], oh_sb[t][:, sl]
                        )

                    return chain_units(
                        ("proj", ib),
                        lambda c: (
                            wo_sb[c][:, 128 * t : 128 * t + 128],
                            ohb_sb[c][:, sl],
                        ),
                        fin,
                    )

                # prefix: just enough for (ib0, pair0, jc0) to start; the fp8
                # nx copy comes after so its DVE work doesn't delay the first
                # k/q PSUM->SBUF copies (and with them the first exp).
                for _, u in k_units(0, 0):
                    u()
                for _, u in q_units(0, 0):
                    u()
                emit_nx8()

                # spare-work queue (ordered; consumed between attention chunks;
                # `ensure` pulls a specific chain's remaining units just-in-time)
                work = []
                # interleave the k0 chunks among the vt chunks so each is a
                # little ahead of its just-in-time `ensure` point in pair 0
                work += vt_units(0, 0) + vt_units(0, 1)
                work += vt_units(1, 0) + vt_units(1, 1) + k_units(0, 1)
                work += vt_units(2, 0) + vt_units(2, 1)
                work += vt_units(3, 0) + vt_units(3, 1) + k_units(0, 2)
                work += vt_units(4, 0) + vt_units(4, 1)
                work += vt_units(5, 0) + vt_units(5, 1) + k_units(0, 3)
                work += vt_units(6, 0) + vt_units(6, 1)
                work += vt_units(7, 0) + vt_units(7, 1)
                # remaining chunks in the order the attention loop needs them
                for t in range(1, CT):
                    work += q_units(t, 0) + k_units(t, 0)
                    work += k_units(t, 1) + k_units(t, 2) + k_units(t, 3)
                for t in range(CT):
                    work += q_units(t, 1)

                def emit_spare(n):
                    for _ in range(n):
                        if work:
                            work.pop(0)[1]()

                def ensure(tag):
                    i = 0
                    while i < len(work):
                        if work[i][0] == tag:
                            work.pop(i)[1]()
                        else:
                            i += 1

                for ib in range(NIB):
                    qsl = slice(512 * ib, 512 * ib + 512)
                    if ib == NIB - 1:
                        for t in range(CT):
                            ensure(("q", t, ib))
                    for pair in range(H // 2):
                        ensure(("q", pair, ib))
                        kt, qt = k_sb[pair], q_sb[pair]
                        ops = pO.tile([128, 2, 512], F32, name="O", tag="O")
                        for j2 in range(NJ2):
                            et = ep.tile([128, 2, 1024], FP8, name="exp", tag="exp")
                            for ko in range(2):
                                jc = 2 * j2 + ko
                                ensure(("k", pair, jc // 4))
                                slots = pS.tile([128, 2, 512], F32, name="S", tag="S")
                                nc.tensor.matmul(
                                    slots[:, 0, :],
                                    kt[0:64, 128 * jc : 128 * jc + 128],
                                    qt[0:64, qsl],
                                    start=True,
                                    stop=True,
                                )
                                nc.tensor.matmul(
                                    slots[:, 1, :],
                                    kt[64:128, 128 * jc : 128 * jc + 128],
                                    qt[64:128, qsl],
                                    start=True,
                                    stop=True,
                                )
                                with nc.allow_low_precision(reason="fp8 softmax intended"):
                                    nc.scalar.activation(
                                        out=et[:, ko, :],
                                        in_=slots[:].rearrange("p a b -> p (a b)"),
                                        func=ACTF.Exp,
                                        scale=float(D) ** -0.5,
                                        bias=ebias_sb[:],
                                    )
                                emit_spare(3 if len(work) > 80 else 2)
                            ensure(("vt", j2))
                            for h01 in range(2):
                                nc.tensor.matmul(
                                    ops[0 : D + 2, h01, :],
                                    vt_sb[j2][:, :, 2 * pair + h01, :],
                                    et[:].rearrange("p a (h n) -> p a h n", n=512)[
                                        :, :, h01, :
                                    ],
                                    start=(j2 == 0),
                                    stop=(j2 == NJ2 - 1),
                                    perf_mode=PMODE.DoubleRow,
                                )
                            emit_spare(1)
                        # ---- pair done: copy O out, normalize in place ----
                        # softmax denominators sit on PSUM partition 64 (ones
                        # column of V^T); reciprocal them there, then a K=1
                        # matmul with a ones row at partition 64 broadcasts
                        # 1/denom down to the head's 64 output channels.  The
                        # broadcast matmul + multiply are deferred into the
                        # NEXT pair's spare stream so they don't head-of-line
                        # block its first S^T matmuls on the PE FIFO.
                        rcps = rp.tile([128, 2, 512], BF16, name="rcps", tag="rcps")
                        with nc.allow_low_precision(reason="bf16 softmax recip"):
                            nc.vector.reciprocal(
                                out=rcps[64:65, :, :], in_=ops[64:65, :, :]
                            )
                        for h01 in range(2):
                            nc.vector.tensor_copy(
                                oh_sb[pair][64 * h01 : 64 * h01 + 64, qsl],
                                ops[0:64, h01, :],
                            )

                        def bm_unit(pair=pair, qsl=qsl, rcps=rcps):
                            bps = pW.tile([128, 512], F32, name="w", tag="w")
                            for h01 in range(2):
                                nc.tensor.matmul(
                                    bps[64 * h01 : 64 * h01 + 64, :],
                                    ones_sb[64:65, 0:64],
                                    rcps[64:65, h01, :],
                                    start=True,
                                    stop=True,
                                )
                            nc.vector.tensor_mul(
                                ohb_sb[pair][:, qsl], oh_sb[pair][:, qsl], bps[:]
                            )

                        if ib == NIB - 1 and pair == H // 2 - 1:
                            bm_unit()
                        else:
                            work.insert(0, (("bm", ib, pair), bm_unit))
                    # ---- ib done: queue (or emit) its projection ----
                    if ib < NIB - 1:
                        for t in range(CT):
                            work += proj_units(t, ib)
                    else:
                        emit_spare(len(work))  # drain any stragglers
                        for t in range(CT):
                            for _, u in proj_units(t, ib):
                                u()

    nc.compile()
    return nc


_NC_CACHE = None


def _get_nc():
    global _NC_CACHE
    if _NC_CACHE is None:
        _NC_CACHE = build_nc()
    return _NC_CACHE


def _host_inputs(x, gn_w, gn_b, w_qkv, w_out, b_out):
    w_qkvT = np.ascontiguousarray(w_qkv.T).astype(ml_dtypes.bfloat16)
    w_outT = np.ascontiguousarray(w_out.T).astype(ml_dtypes.bfloat16)
    wv8 = np.ascontiguousarray(
        np.asarray(w_qkv[2 * C : 3 * C, :].T, np.float32)
        .reshape(CT, 128, C)
        .transpose(1, 0, 2)
    ).astype(ml_dtypes.float8_e4m3fn)
    ident = np.eye(128, dtype=np.float32)
    shared = {
        "wqkvT": w_qkvT,
        "wv8": wv8,
        "woutT": w_outT,
        "gnw": np.ascontiguousarray(gn_w.reshape(CT, 128), np.float32),
        "gnb": np.ascontiguousarray(gn_b.reshape(CT, 128), np.float32),
        "bout": np.ascontiguousarray(b_out.reshape(CT, 128).T, np.float32),
        "ident": ident,
    }
    in_maps = []
    for core in range(8):
        b, ih = core // 2, core % 2
        xb = np.asarray(x[b], np.float32)
        if ih:
            xb = np.concatenate([xb[:, LQ:], xb[:, :LQ]], axis=1)
        in_maps.append({"x": np.ascontiguousarray(xb), **shared})
    return in_maps


def kernel(x, gn_w, gn_b, w_qkv, w_out, b_out):
    nc = _get_nc()
    in_maps = _host_inputs(
        np.asarray(x), np.asarray(gn_w), np.asarray(gn_b),
        np.asarray(w_qkv), np.asarray(w_out), np.asarray(b_out),
    )
    res = run_bass_kernel_spmd(nc, in_maps, list(range(8)))
    y = np.empty((B, C, L), np.float32)
    for core in range(8):
        b, ih = core // 2, core % 2
        y[b][:, ih * LQ : (ih + 1) * LQ] = res.results[core]["y"]
    return y


# revision 48
# speedup vs baseline: 1.8230x; 1.0298x over previous
"""AttentionBlock (GroupNorm + 8-head self-attention + out-proj + residual) on 8 trn2 cores.

Sharding: core = (batch b, query-half ih).  Each core gets x[b] rolled so that
"its" 1024 query positions are columns 0:1024; K/V are computed over the full
(rolled) L=2048, which is sound because attention and the group-norm statistics
are invariant to a permutation of key/value positions.  Output is the core's
[512, 1024] slice of proj + residual; the host reassembles [4, 512, 2048].

Structure (v2, ACT-saturating):
 - The softmax exp is the hard floor (~133us of ACT time per core: 16.8M
   elements at 1 elem/cycle/lane @1.2GHz + per-inst bubble).  Everything else
   is organized to hide under a continuous stream of 128 exp instructions.
 - exp reads S^T straight from PSUM ([128 keys, 2x512 cols] per 128-key chunk)
   and writes fp8e4m3 to SBUF with the 1/8 scale and a -3 logit bias folded in
   (softmax is shift-invariant; keeps exp < 448 = fp8 max).
 - O = V^T-augmented @ exp runs in fp8 with perf_mode=DoubleRow: 256 keys per
   MM via the [128, 2, *] k-interleave, half the PE time of bf16.  A ones
   column in V^T produces the softmax denominator on PSUM partition 64.
 - The V^T = nx^T Wv matmuls also run fp8 DoubleRow (256 channels per MM) off
   a dedicated fp8 copy of nx, so pair 0 can feed V^T just-in-time without
   starving the exp stream.
 - All other PE work (remaining qkv tiles, V^T chunks, out-proj) is emitted as
   single-matmul "spare work" units interleaved between attention chunks (a
   tagged queue with just-in-time `ensure` pulls), so the PE never bursts long
   enough to starve ACT.
 - Softmax division: 1/denominator is computed in place on PSUM partition 64,
   then a K=1 matmul against a ones row at partition 64 broadcasts it to the
   head's 64 channels - no cross-partition DMA, normalize is per-pair local.
 - ACT does nothing else: group-norm rstd uses Ln+Exp (same ACT table set as
   the softmax exp; a dummy Ln at kernel start hoists the table load off the
   critical path), one nx tile is applied via ACT activation(Identity) to
   shorten the DVE prefix, and bulk DMAs go on SP/Pool queues.
 - PSUM: 4 banks S^T (2 rotations), 2 banks O accum, 2 banks qkv/proj chains.
"""

import sys

sys.path.insert(0, "/opt/trn_rl_repo")

import numpy as np
import ml_dtypes

import concourse.bass as bass
import concourse.mybir as mybir
import concourse.tile as tile
from concourse import bacc
from concourse.vector_clock import ScopedClock, VectorClock
from concourse.bass_utils import run_bass_kernel_spmd

F32 = mybir.dt.float32
BF16 = mybir.dt.bfloat16
FP8 = mybir.dt.float8e4
AX = mybir.AxisListType
OP = mybir.AluOpType
ACTF = mybir.ActivationFunctionType
PMODE = mybir.MatmulPerfMode

B, C, L = 4, 512, 2048
H, D = 8, 64
G, EPS = 32, 1e-5
LQ = L // 2          # queries per core
CT = C // 128        # channel tiles
NJC = L // 128       # key chunks of 128
NJ2 = NJC // 2       # key double-chunks of 256 (DoubleRow granularity)
NIB = LQ // 512      # 512-wide query blocks
DP = D + 2           # V^T head stride: 64 V cols + ones col + zero pad (66, so 8*66 % 16 == 0)
EXP_BIAS = -3.0      # exp(s/8 - 3): softmax-invariant shift, keeps fp8 in range


class _SplitDrainTC(tile.TileContext):
    """Stock exit puts every outstanding proc's wait on one SP Drain; this
    walrus build caps sync-waits per instruction, so spread them over
    single-wait NOPs first."""

    def _drain_and_barrier(self, tick_clock, wait_clock):
        g = tick_clock.global_clock
        for proc in range(len(g)):
            if g[proc] == 0:
                continue
            vc = VectorClock([0] * len(g))
            vc.require_at_least(proc, g[proc])
            nop = self.nc.sync.nop(hint=f"split_drain_{proc}")
            wait_clock.add_sem_waits(nop.ins, ScopedClock({None: vc}))
        self.nc.sync.drain()
        self.nc.all_engine_barrier()
        assert self.sems is not None
        popped = self.nc._tile_sem_poison_stack.pop()
        assert popped is self._sem_poison
        self.nc.clear_and_free_semaphores(list(self.sems.allocated().values()))
        self.nc.all_engine_barrier()


def build_nc(reps: int = 1):
    nc = bacc.Bacc("TRN2", target_bir_lowering=False, num_devices=8)

    xd = nc.declare_dram_parameter("x", [C, L], F32, isOutput=False)
    wqkvT = nc.declare_dram_parameter("wqkvT", [C, 3 * C], BF16, isOutput=False)
    wv8d = nc.declare_dram_parameter("wv8", [128, CT, C], FP8, isOutput=False)
    woutT = nc.declare_dram_parameter("woutT", [C, C], BF16, isOutput=False)
    gnwd = nc.declare_dram_parameter("gnw", [CT, 128], F32, isOutput=False)
    gnbd = nc.declare_dram_parameter("gnb", [CT, 128], F32, isOutput=False)
    boutd = nc.declare_dram_parameter("bout", [128, CT], F32, isOutput=False)
    identd = nc.declare_dram_parameter("ident", [128, 128], F32, isOutput=False)
    yd = nc.declare_dram_parameter("y", [C, LQ], F32, isOutput=True)

    import contextlib

    with _SplitDrainTC(nc) as tc:
        with (
            tc.For_i(0, reps, 1) if reps > 1 else contextlib.nullcontext()
        ), tc.tile_pool(name="persist", bufs=1) as pp:
            x_sb = [pp.tile([128, L], F32, name=f"x{t}", tag=f"x{t}") for t in range(CT)]
            wq_sb = [pp.tile([128, 3 * C], BF16, name=f"wq{t}", tag=f"wq{t}") for t in range(CT)]
            wo_sb = [pp.tile([128, C], BF16, name=f"wo{t}", tag=f"wo{t}") for t in range(CT)]
            nx_sb = [pp.tile([128, L], BF16, name=f"nx{t}", tag=f"nx{t}") for t in range(CT)]
            # fp8 copy of nx (single tensor so DoubleRow APs can span channel
            # tile pairs) + fp8 v-weights: the V^T matmul runs fp8 DoubleRow.
            nx8_sb = pp.tile([128, CT, L], FP8, name="nx8", tag="nx8")
            wv8_sb = pp.tile([128, CT, C], FP8, name="wv8", tag="wv8")
            q_sb = [pp.tile([128, LQ], BF16, name=f"q{t}", tag=f"q{t}") for t in range(CT)]
            k_sb = [pp.tile([128, L], BF16, name=f"k{t}", tag=f"k{t}") for t in range(CT)]
            # V^T double-chunks: [key-in-chunk, ko, head*DP + d]; d=64 is the
            # ones column (denominator), d=65 zero padding.
            vt_sb = [
                pp.tile([128, 2, H, DP], FP8, name=f"vt{j}", tag=f"vt{j}")
                for j in range(NJ2)
            ]
            oh_sb = [pp.tile([128, LQ], F32, name=f"oh{t}", tag=f"oh{t}") for t in range(CT)]
            ohb_sb = [pp.tile([128, LQ], BF16, name=f"ohb{t}", tag=f"ohb{t}") for t in range(CT)]
            gnw_sb = pp.tile([CT, 128], F32, name="gnw", tag="gnw")
            gnb_sb = pp.tile([CT, 128], F32, name="gnb", tag="gnb")
            bout_sb = pp.tile([128, CT], F32, name="bout", tag="bout")
            ones_sb = pp.tile([128, 64], BF16, name="ones", tag="ones")
            nc.vector.memset(ones_sb[:], 1.0)
            ident_sb = pp.tile([128, 128], F32, name="ident", tag="ident")
            sparam_sb = pp.tile([128, 3, CT], F32, name="sparam", tag="sparam")
            ebias_sb = pp.tile([128, 1], F32, name="ebias", tag="ebias")
            nc.vector.memset(ebias_sb[:], EXP_BIAS)
            dummy_sb = pp.tile([1, 1], F32, name="dummy", tag="dummy")
            nc.vector.memset(dummy_sb[:], 1.0)

            # ---------------- input DMA (SP + Pool only; ACT stays clean) ----
            nc.gpsimd.dma_start(ident_sb[:], identd[:])
            nc.gpsimd.dma_start(gnw_sb[:], gnwd[:])
            nc.gpsimd.dma_start(gnb_sb[:], gnbd[:])
            nc.gpsimd.dma_start(bout_sb[:], boutd[:])
            xdma_engs = [nc.sync, nc.gpsimd, nc.scalar]
            di = 0
            for t in range(CT):
                for sg in range(4):
                    csl = slice(512 * sg, 512 * sg + 512)
                    xdma_engs[di % 3].dma_start(
                        x_sb[t][:, csl], xd[128 * t : 128 * t + 128, csl]
                    )
                    di += 1
            dma_engs = [nc.sync, nc.gpsimd]
            for t in range(CT):
                for half in range(2):
                    wsl = slice(768 * half, 768 * half + 768)
                    dma_engs[di % 2].dma_start(
                        wq_sb[t][:, wsl], wqkvT[128 * t : 128 * t + 128, wsl]
                    )
                    di += 1
                dma_engs[di % 2].dma_start(
                    wo_sb[t][:], woutT[128 * t : 128 * t + 128, :]
                )
                di += 1
            nc.sync.dma_start(wv8_sb[:], wv8d[:])
            # dummy Ln pulls the natural_log_exp table load (1.3us) off the
            # stats critical path: it happens here while ACT is otherwise idle
            nc.scalar.activation(out=dummy_sb[:], in_=dummy_sb[:], func=ACTF.Ln)

            # ---------------- group norm statistics ----------------
            with (
                tc.tile_pool(name="gtmp", bufs=2) as gp,
                tc.tile_pool(name="gps", bufs=2, space="PSUM") as gpp,
            ):
                # stats_all col t = channel-mean(tile t), col 32+t = channel-var:
                # after PE transpose, means land on partitions 0..3 and vars on
                # 32..35 (engine APs may only start at partition 0/32/64/96).
                stats_all = gp.tile([128, 36], F32, name="stats_all", tag="stats_all")
                nc.vector.memset(stats_all[:], 0.0)
                # tile 3's stats go through ACT (otherwise idle): sum(x) and
                # sum(x^2) via activation accum_out, shortening the DVE
                # bn_stats chain that gates the whole prefix.
                sq_scr = gp.tile([128, L], F32, name="sq_scr", tag="sq_scr")
                sxa = gp.tile([128, 2], F32, name="sxa", tag="sxa")
                nc.scalar.activation(
                    out=sq_scr[:], in_=x_sb[3][:], func=ACTF.Identity,
                    accum_out=sxa[:, 0:1],
                )
                nc.scalar.activation(
                    out=sq_scr[:], in_=x_sb[3][:], func=ACTF.Square,
                    accum_out=sxa[:, 1:2],
                )
                for t in range(3):
                    st6 = gp.tile([128, 4, 6], F32, name="st6", tag="st6")
                    for sg in range(4):
                        nc.vector.bn_stats(
                            out=st6[:, sg, :],
                            in_=x_sb[t][:, 512 * sg : 512 * sg + 512],
                        )
                    sa = stats_all[:]
                    mv_out = bass.AP(
                        tensor=sa.tensor, offset=sa.offset + t, ap=[sa.ap[0], [32, 2]]
                    )
                    nc.vector.bn_aggr(out=mv_out, in_=st6[:])
                # stats_all[:, 3] = mean3, stats_all[:, 35] = var3
                nc.vector.tensor_scalar_mul(stats_all[:, 3:4], sxa[:, 0:1], 1.0 / L)
                nc.vector.tensor_scalar_mul(stats_all[:, 35:36], sxa[:, 1:2], 1.0 / L)
                m3sq = gp.tile([128, 1], F32, name="m3sq", tag="m3sq")
                nc.vector.tensor_mul(m3sq[:], stats_all[:, 3:4], stats_all[:, 3:4])
                nc.vector.tensor_sub(
                    stats_all[:, 35:36], stats_all[:, 35:36], m3sq[:]
                )

                st_ps = gpp.tile([36, 128], F32, name="st_ps", tag="st_ps")
                nc.tensor.transpose(st_ps[:], stats_all[:], ident_sb[:])
                statsT = gp.tile([36, 128], F32, name="statsT", tag="statsT")
                nc.vector.tensor_copy(statsT[:], st_ps[:])

                mred = gp.tile([4, 8], F32, name="mred", tag="mred")
                nc.vector.tensor_reduce(
                    out=mred[:],
                    in_=statsT[0:4, :].rearrange("p (g s) -> p g s", s=16),
                    axis=AX.X,
                    op=OP.add,
                )
                vred = gp.tile([4, 8], F32, name="vred", tag="vred")
                nc.vector.tensor_reduce(
                    out=vred[:],
                    in_=statsT[32:36, :].rearrange("p (g s) -> p g s", s=16),
                    axis=AX.X,
                    op=OP.add,
                )
                sq = gp.tile([4, 128], F32, name="sq", tag="sq")
                nc.vector.tensor_mul(sq[:], statsT[0:4, :], statsT[0:4, :])
                sqred = gp.tile([4, 8], F32, name="sqred", tag="sqred")
                nc.vector.tensor_reduce(
                    out=sqred[:],
                    in_=sq[:].rearrange("p (g s) -> p g s", s=16),
                    axis=AX.X,
                    op=OP.add,
                )
                mg = gp.tile([4, 8], F32, name="mg", tag="mg")
                nc.vector.tensor_scalar_mul(mg[:], mred[:], 1.0 / 16)
                # vg = red_var/16 + sqred/16 - mg^2
                vg = gp.tile([4, 8], F32, name="vg", tag="vg")
                nc.vector.tensor_scalar_mul(vg[:], vred[:], 1.0 / 16)
                nc.vector.scalar_tensor_tensor(
                    out=vg[:],
                    in0=sqred[:],
                    scalar=1.0 / 16,
                    in1=vg[:],
                    op0=OP.mult,
                    op1=OP.add,
                )
                mg2 = gp.tile([4, 8], F32, name="mg2", tag="mg2")
                nc.vector.tensor_mul(mg2[:], mg[:], mg[:])
                nc.vector.tensor_sub(vg[:], vg[:], mg2[:])
                # rstd = (vg + eps)^-0.5 = exp(-0.5*ln(vg + eps)); Log and Exp
                # share one ACT table set with the softmax exp, so the kernel
                # pays exactly one table load.
                epst = gp.tile([4, 1], F32, name="epst", tag="epst")
                nc.vector.memset(epst[:], EPS)
                lvg = gp.tile([4, 8], F32, name="lvg", tag="lvg")
                nc.scalar.activation(out=lvg[:], in_=vg[:], func=ACTF.Ln, bias=epst[:])
                nc.scalar.activation(out=vg[:], in_=lvg[:], func=ACTF.Exp, scale=-0.5)

                # broadcast group -> channels: [4, 8] -> [4, 128]
                def bcast16(src):
                    a = src.ap
                    return bass.AP(
                        tensor=src.tensor, offset=src.offset, ap=[a[0], a[1], [0, 16]]
                    )

                rstd_bc = gp.tile([4, 128], F32, name="rstd_bc", tag="rstd_bc")
                nc.vector.tensor_copy(
                    rstd_bc[:].rearrange("p (g s) -> p g s", s=16), bcast16(vg[:])
                )
                mg_bc = gp.tile([4, 128], F32, name="mg_bc", tag="mg_bc")
                nc.vector.tensor_copy(
                    mg_bc[:].rearrange("p (g s) -> p g s", s=16), bcast16(mg[:])
                )
                s2 = gp.tile([4, 128], F32, name="s2", tag="s2")
                nc.vector.tensor_mul(s2[:], rstd_bc[:], gnw_sb[0:4, :])
                s1 = gp.tile([4, 128], F32, name="s1", tag="s1")
                nc.vector.reciprocal(out=s1[:], in_=s2[:])
                nc.vector.tensor_mul(s1[:], s1[:], gnb_sb[0:4, :])
                nc.vector.tensor_sub(s1[:], mg_bc[:], s1[:])

                # third column: -(s1*s2), the bias form ACT's activation needs
                # for nx = x*s2 + (-s1*s2)
                s12 = gp.tile([4, 128], F32, name="s12", tag="s12")
                nc.vector.scalar_tensor_tensor(
                    out=s12[:], in0=s1[:], scalar=-1.0, in1=s2[:],
                    op0=OP.mult, op1=OP.mult,
                )
                sp_ps = gpp.tile([128, 3, CT], F32, name="sp_ps", tag="sp_ps")
                nc.tensor.transpose(sp_ps[:, 0, :], s1[:], ident_sb[0:4, 0:4])
                nc.tensor.transpose(sp_ps[:, 1, :], s2[:], ident_sb[0:4, 0:4])
                nc.tensor.transpose(sp_ps[:, 2, :], s12[:], ident_sb[0:4, 0:4])
                nc.vector.tensor_copy(sparam_sb[:], sp_ps[:])

            # group-norm apply: nx = (x - s1) * s2, cast to bf16 (+ fp8 for V).
            # Tile 3 goes through ACT (idle here) so the DVE finishes sooner.
            for t in range(3):
                nc.vector.tensor_scalar(
                    out=nx_sb[t][:],
                    in0=x_sb[t][:],
                    scalar1=sparam_sb[:, 0, t : t + 1],
                    scalar2=sparam_sb[:, 1, t : t + 1],
                    op0=OP.subtract,
                    op1=OP.mult,
                )
            nc.scalar.activation(
                out=nx_sb[3][:],
                in_=x_sb[3][:],
                func=ACTF.Identity,
                scale=sparam_sb[:, 1, 3:4],
                bias=sparam_sb[:, 2, 3:4],
            )
            def emit_nx8():
                with nc.allow_low_precision(reason="fp8 V-path intended"):
                    for t in range(CT):
                        nc.vector.tensor_scalar(
                            out=nx8_sb[:, t, :],
                            in0=x_sb[t][:],
                            scalar1=sparam_sb[:, 0, t : t + 1],
                            scalar2=sparam_sb[:, 1, t : t + 1],
                            op0=OP.subtract,
                            op1=OP.mult,
                        )

            # ---------------- attention + interleaved qkv/proj ----------------
            with (
                tc.tile_pool(name="psS", bufs=2, space="PSUM") as pS,
                tc.tile_pool(name="psO", bufs=1, space="PSUM") as pO,
                tc.tile_pool(name="psW", bufs=2, space="PSUM") as pW,
                tc.tile_pool(name="expp", bufs=3) as ep,
                tc.tile_pool(name="rcpp", bufs=2) as rp,
            ):
                # Spare work is sliced into single-matmul units so the PE
                # stream between attention chunks never bursts long enough to
                # delay the next S^T matmul (which gates the exp stream).
                def chain_units(tag, mm_args, finish, n=CT, perf_mode=None):
                    cell = {}
                    units = []

                    def mk(c):
                        def u():
                            if c == 0:
                                cell["ps"] = pW.tile([128, 512], F32, name="w", tag="w")
                            lhsT, rhs = mm_args(c)
                            nc.tensor.matmul(
                                cell["ps"][:], lhsT, rhs,
                                start=(c == 0), stop=(c == n - 1),
                                perf_mode=perf_mode,
                            )
                            if c == n - 1:
                                finish(cell["ps"])
                        return u

                    return [(tag, mk(c)) for c in range(n)]

                def k_units(t, nb):
                    def fin(ps):
                        nc.vector.tensor_copy(
                            k_sb[t][:, 512 * nb : 512 * nb + 512], ps[:]
                        )
                    return chain_units(
                        ("k", t, nb),
                        lambda c: (
                            wq_sb[c][:, C + 128 * t : C + 128 * t + 128],
                            nx_sb[c][:, 512 * nb : 512 * nb + 512],
                        ),
                        fin,
                    )

                def q_units(t, nb):
                    def fin(ps):
                        nc.vector.tensor_copy(
                            q_sb[t][:, 512 * nb : 512 * nb + 512], ps[:]
                        )
                    return chain_units(
                        ("q", t, nb),
                        lambda c: (
                            wq_sb[c][:, 128 * t : 128 * t + 128],
                            nx_sb[c][:, 512 * nb : 512 * nb + 512],
                        ),
                        fin,
                    )

                def vt_units(j2, ko):
                    lt = 2 * j2 + ko

                    def fin(ps):
                        with nc.allow_low_precision(reason="fp8 attention intended"):
                            if ko == 0:
                                nc.vector.memset(vt_sb[j2][:, :, :, D : D + 1], 1.0)
                                nc.vector.memset(vt_sb[j2][:, :, :, D + 1 : D + 2], 0.0)
                            nc.vector.tensor_copy(
                                vt_sb[j2][:, ko, :, 0:D],
                                ps[:].rearrange("p (h d) -> p h d", d=D),
                            )

                    # fp8 DoubleRow over channel-tile pairs: 2 MMs instead of 4
                    return chain_units(
                        ("vt", j2),
                        lambda u: (
                            nx8_sb[:, 2 * u : 2 * u + 2, 128 * lt : 128 * lt + 128],
                            wv8_sb[:, 2 * u : 2 * u + 2, :],
                        ),
                        fin,
                        n=2,
                        perf_mode=PMODE.DoubleRow,
                    )

                def proj_units(t, ib):
                    sl = slice(512 * ib, 512 * ib + 512)

                    def fin(ps):
                        # y = (proj + b_out) + residual, fused; reuse oh as staging
                        nc.vector.scalar_tensor_tensor(
                            out=oh_sb[t][:, sl],
                            in0=ps[:],
                            scalar=bout_sb[:, t : t + 1],
                            in1=x_sb[t][:, sl],
                            op0=OP.add,
                            op1=OP.add,
                        )
                        nc.sync.dma_start(
                            yd[128 * t : 128 * t + 128, sl], oh_sb[t][:, sl]
                        )

                    return chain_units(
                        ("proj", ib),
                        lambda c: (
                            wo_sb[c][:, 128 * t : 128 * t + 128],
                            ohb_sb[c][:, sl],
                        ),
                        fin,
                    )

                # prefix: just enough for (ib0, pair0, jc0) to start; the fp8
                # nx copy comes after so its DVE work doesn't delay the first
                # k/q PSUM->SBUF copies (and with them the first exp).
                for _, u in k_units(0, 0):
                    u()
                for _, u in q_units(0, 0):
                    u()
                emit_nx8()

                # spare-work queue (ordered; consumed between attention chunks;
                # `ensure` pulls a specific chain's remaining units just-in-time)
                work = []
                # interleave the k0 chunks among the vt chunks so each is a
                # little ahead of its just-in-time `ensure` point in pair 0
                work += vt_units(0, 0) + vt_units(0, 1)
                work += vt_units(1, 0) + vt_units(1, 1) + k_units(0, 1)
                work += vt_units(2, 0) + vt_units(2, 1)
                work += vt_units(3, 0) + vt_units(3, 1) + k_units(0, 2)
                work += vt_units(4, 0) + vt_units(4, 1)
                work += vt_units(5, 0) + vt_units(5, 1) + k_units(0, 3)
                work += vt_units(6, 0) + vt_units(6, 1)
                work += vt_units(7, 0) + vt_units(7, 1)
                # remaining chunks in the order the attention loop needs them
                for t in range(1, CT):
                    work += q_units(t, 0) + k_units(t, 0)
                    work += k_units(t, 1) + k_units(t, 2) + k_units(t, 3)
                for t in range(CT):
                    work += q_units(t, 1)

                def emit_spare(n):
                    for _ in range(n):
                        if work:
                            work.pop(0)[1]()

                def ensure(tag):
                    i = 0
                    while i < len(work):
                        if work[i][0] == tag:
                            work.pop(i)[1]()
                        else:
                            i += 1

                for ib in range(NIB):
                    qsl = slice(512 * ib, 512 * ib + 512)
                    if ib == NIB - 1:
                        for t in range(CT):
                            ensure(("q", t, ib))
                    for pair in range(H // 2):
                        ensure(("q", pair, ib))
                        kt, qt = k_sb[pair], q_sb[pair]
                        ops = pO.tile([128, 2, 512], F32, name="O", tag="O")
                        for j2 in range(NJ2):
                            et = ep.tile([128, 2, 1024], FP8, name="exp", tag="exp")
                            for ko in range(2):
                                jc = 2 * j2 + ko
                                ensure(("k", pair, jc // 4))
                                slots = pS.tile([128, 2, 512], F32, name="S", tag="S")
                                nc.tensor.matmul(
                                    slots[:, 0, :],
                                    kt[0:64, 128 * jc : 128 * jc + 128],
                                    qt[0:64, qsl],
                                    start=True,
                                    stop=True,
                                )
                                nc.tensor.matmul(
                                    slots[:, 1, :],
                                    kt[64:128, 128 * jc : 128 * jc + 128],
                                    qt[64:128, qsl],
                                    start=True,
                                    stop=True,
                                )
                                with nc.allow_low_precision(reason="fp8 softmax intended"):
                                    nc.scalar.activation(
                                        out=et[:, ko, :],
                                        in_=slots[:].rearrange("p a b -> p (a b)"),
                                        func=ACTF.Exp,
                                        scale=float(D) ** -0.5,
                                        bias=ebias_sb[:],
                                    )
                                emit_spare(3 if len(work) > 80 else 2)
                            ensure(("vt", j2))
                            for h01 in range(2):
                                nc.tensor.matmul(
                                    ops[0 : D + 2, h01, :],
                                    vt_sb[j2][:, :, 2 * pair + h01, :],
                                    et[:].rearrange("p a (h n) -> p a h n", n=512)[
                                        :, :, h01, :
                                    ],
                                    start=(j2 == 0),
                                    stop=(j2 == NJ2 - 1),
                                    perf_mode=PMODE.DoubleRow,
                                )
                            emit_spare(1)
                        # ---- pair done: copy O out, normalize in place ----
                        # softmax denominators sit on PSUM partition 64 (ones
                        # column of V^T); reciprocal them there, then a K=1
                        # matmul with a ones row at partition 64 broadcasts
                        # 1/denom down to the head's 64 output channels.  The
                        # broadcast matmul + multiply are deferred into the
                        # NEXT pair's spare stream so they don't head-of-line
                        # block its first S^T matmuls on the PE FIFO.
                        rcps = rp.tile([128, 2, 512], BF16, name="rcps", tag="rcps")
                        with nc.allow_low_precision(reason="bf16 softmax recip"):
                            nc.vector.reciprocal(
                                out=rcps[64:65, :, :], in_=ops[64:65, :, :]
                            )
                        for h01 in range(2):
                            nc.vector.tensor_copy(
                                oh_sb[pair][64 * h01 : 64 * h01 + 64, qsl],
                                ops[0:64, h01, :],
                            )

                        def bm_unit(pair=pair, qsl=qsl, rcps=rcps):
                            bps = pW.tile([128, 512], F32, name="w", tag="w")
                            for h01 in range(2):
                                nc.tensor.matmul(
                                    bps[64 * h01 : 64 * h01 + 64, :],
                                    ones_sb[64:65, 0:64],
                                    rcps[64:65, h01, :],
                                    start=True,
                                    stop=True,
                                )
                            nc.vector.tensor_mul(
                                ohb_sb[pair][:, qsl], oh_sb[pair][:, qsl], bps[:]
                            )

                        if ib == NIB - 1 and pair == H // 2 - 1:
                            bm_unit()
                        else:
                            work.insert(0, (("bm", ib, pair), bm_unit))
                    # ---- ib done: queue (or emit) its projection ----
                    if ib < NIB - 1:
                        for t in range(CT):
                            work += proj_units(t, ib)
                    else:
                        emit_spare(len(work))  # drain any stragglers
                        for t in range(CT):
                            for _, u in proj_units(t, ib):
                                u()

    nc.compile()
    return nc


_NC_CACHE = None


def _get_nc():
    global _NC_CACHE
    if _NC_CACHE is None:
        _NC_CACHE = build_nc()
    return _NC_CACHE


def _host_inputs(x, gn_w, gn_b, w_qkv, w_out, b_out):
    w_qkvT = np.ascontiguousarray(w_qkv.T).astype(ml_dtypes.bfloat16)
    w_outT = np.ascontiguousarray(w_out.T).astype(ml_dtypes.bfloat16)
    wv8 = np.ascontiguousarray(
        np.asarray(w_qkv[2 * C : 3 * C, :].T, np.float32)
        .reshape(CT, 128, C)
        .transpose(1, 0, 2)
    ).astype(ml_dtypes.float8_e4m3fn)
    ident = np.eye(128, dtype=np.float32)
    shared = {
        "wqkvT": w_qkvT,
        "wv8": wv8,
        "woutT": w_outT,
        "gnw": np.ascontiguousarray(gn_w.reshape(CT, 128), np.float32),
        "gnb": np.ascontiguousarray(gn_b.reshape(CT, 128), np.float32),
        "bout": np.ascontiguousarray(b_out.reshape(CT, 128).T, np.float32),
        "ident": ident,
    }
    in_maps = []
    for core in range(8):
        b, ih = core // 2, core % 2
        xb = np.asarray(x[b], np.float32)
        if ih:
            xb = np.concatenate([xb[:, LQ:], xb[:, :LQ]], axis=1)
        in_maps.append({"x": np.ascontiguousarray(xb), **shared})
    return in_maps


def kernel(x, gn_w, gn_b, w_qkv, w_out, b_out):
    nc = _get_nc()
    in_maps = _host_inputs(
        np.asarray(x), np.asarray(gn_w), np.asarray(gn_b),
        np.asarray(w_qkv), np.asarray(w_out), np.asarray(b_out),
    )
    res = run_bass_kernel_spmd(nc, in_maps, list(range(8)))
    y = np.empty((B, C, L), np.float32)
    for core in range(8):
        b, ih = core // 2, core % 2
        y[b][:, ih * LQ : (ih + 1) * LQ] = res.results[core]["y"]
    return y
